# revision 1
# baseline (speedup 1.0000x reference)
"""AttentionBlock (GroupNorm + single-head attention over HW tokens + proj +
residual) as a Bass/Tile kernel for 8 Trainium2 NeuronCores.

Sharding: data-parallel over batch B=32 -> 4 samples per core; 1x1-conv
weights replicated.

Per-sample dataflow on one core (C=256, HW=1024, fp32 in / fp32 out,
float32r matmuls; ~121us modeled / ~116-120us measured per kernel):
  GroupNorm: per-channel sum/sumsq via ACT Copy/Square accum (chunk 0) and
    DVE reduce/scalar_tensor_tensor accum (chunk 1) in parallel; group sums
    via tiny fp32 matmul against a 1/n-scaled group mask; rstd = DVE Newton
    rsqrt (bit-trick seed, 2 iters, ACT stays on the exp table set);
    per-channel scale/shift broadcast via tiny matmul against a gamma-scaled
    maskT; h = x*sc + sh (float32r). All stat pipelines are emitted 2 samples
    ahead (prologue / mid-attention) so steady-state streams have no stat
    dependencies.
  QKV: q,k in (C, HW) layout (lhsT = w^T chunks, PSUM->SBUF copies with bias
    on ACT); vT in (HW, C) layout (lhsT = h chunks) with PAIRS of chunks
    sharing one PSUM bank (only the first matmul uses start=True) so one DVE
    copy drains two. 1/sqrt(C) folded into w_q/b_q on the host.
  Scores transposed: sT[j,i] = sum_c k[c,j] q[c,i]; softmax over j
    (= partitions) without max-subtraction (scores are O(6), exp is safe in
    fp32): pT = exp(sT) on ACT; denominators via ones-column matmuls over pT,
    reciprocal broadcast across partitions via a rank-1 ones-row matmul.
  attnout[c,i] = sum_j vT[j,c] pT[j,i] (lhsT = vT chunks), v-bias folded in
    as per-partition add on the ACT PSUM->SBUF copy.
  proj per query-half: lhsT = w_proj^T chunks over attnout; final out =
    (proj*recip + b_proj) + x in two DVE passes, DMA out.
"""

import numpy as np

import concourse.bacc as bacc
import concourse.tile as tile
import concourse.mybir as mybir
from concourse.bass_utils import run_bass_kernel_spmd

F32 = mybir.dt.float32
F32R = mybir.dt.float32r
ALU = mybir.AluOpType
ACTF = mybir.ActivationFunctionType

N_CORES = 8
B, C, H, W = 32, 256, 32, 32
HW = H * W          # 1024
S = B // N_CORES    # 4 samples per core
G = 8               # groups
CG = C // G         # 32 channels per group
EPS = 1e-5
NC2 = C // 128      # channel chunks of 128
NH2 = HW // 512     # hw halves of 512


def _emit_stats(nc, pools, wt, s, x_ap, post_x_cb=None):
    """Load x and run the whole GroupNorm scalar pipeline down to per-channel
    scale/shift columns. Emitted in the kernel prologue for every sample so
    the steady-state engine streams carry no stat dependencies. Stats run on
    ACT (Copy/Square + accum, same table set as exp); inv_n is folded into
    the group-sum mask, gamma into the broadcast mask, rsqrt is a DVE
    Newton iteration."""
    sb, ps = pools
    I32 = mybir.dt.int32

    xt, st = [], []
    for ci in range(NC2):
        x_t = sb.tile([128, HW], F32, name=f"x_s{s}c{ci}", tag=f"x{ci}", bufs=S)
        if s == 0:
            # sample 0 is on the kernel-start critical path: DMA in halves so
            # stats overlap the transfer, partial accums combined below
            nc.sync.dma_start(x_t[:, 0:512],
                              x_ap[s, ci * 128:(ci + 1) * 128, 0:512])
            nc.sync.dma_start(x_t[:, 512:HW],
                              x_ap[s, ci * 128:(ci + 1) * 128, 512:HW])
        else:
            nc.sync.dma_start(x_t[:], x_ap[s, ci * 128:(ci + 1) * 128, :])
        xt.append(x_t)
    if post_x_cb is not None:
        post_x_cb()
    for ci in range(NC2):
        x_t = xt[ci]
        st_t = sb.tile([128, 2], F32, name=f"st_s{s}c{ci}", tag=f"st{ci}", bufs=S)
        if s == 0:
            p4 = sb.tile([128, 4], F32, name=f"p4_s{s}c{ci}", tag=f"p4{ci}",
                         bufs=1)
            for hh in range(2):
                xs = x_t[:, hh * 512:(hh + 1) * 512]
                if ci == 0:
                    scr = sb.tile([128, 512], F32, name=f"scr_s{s}c{ci}e{hh}",
                                  tag="scrh", bufs=2)
                    nc.scalar.activation(scr[:], xs, ACTF.Copy,
                                         accum_out=p4[:, hh:hh + 1])
                    scr2 = sb.tile([128, 512], F32, name=f"sq_s{s}c{ci}e{hh}",
                                   tag="scrh", bufs=2)
                    nc.scalar.activation(scr2[:], xs, ACTF.Square,
                                         accum_out=p4[:, 2 + hh:3 + hh])
                else:
                    nc.vector.reduce_sum(p4[:, hh:hh + 1], xs,
                                         axis=mybir.AxisListType.X)
                    scr3 = sb.tile([128, 512], F32, name=f"sq_s{s}c{ci}e{hh}",
                                   tag="scrh", bufs=2)
                    nc.vector.scalar_tensor_tensor(
                        scr3[:], in0=xs, scalar=0.0, in1=xs, op0=ALU.add,
                        op1=ALU.mult, accum_out=p4[:, 2 + hh:3 + hh])
            nc.vector.tensor_add(st_t[:, 0:1], p4[:, 0:1], p4[:, 1:2])
            nc.vector.tensor_add(st_t[:, 1:2], p4[:, 2:3], p4[:, 3:4])
        elif ci == 0:
            scr = sb.tile([128, HW], F32, name=f"scr_s{s}c{ci}", tag="scr", bufs=2)
            nc.scalar.activation(scr[:], x_t[:], ACTF.Copy,
                                 accum_out=st_t[:, 0:1])
            scr2 = sb.tile([128, HW], F32, name=f"sq_s{s}c{ci}", tag="scr", bufs=2)
            nc.scalar.activation(scr2[:], x_t[:], ACTF.Square,
                                 accum_out=st_t[:, 1:2])
        else:
            nc.vector.reduce_sum(st_t[:, 0:1], x_t[:], axis=mybir.AxisListType.X)
            scr3 = sb.tile([128, HW], F32, name=f"sq_s{s}c{ci}", tag="scr", bufs=2)
            nc.vector.scalar_tensor_tensor(scr3[:], in0=x_t[:], scalar=0.0,
                                           in1=x_t[:], op0=ALU.add,
                                           op1=ALU.mult,
                                           accum_out=st_t[:, 1:2])
        st.append(st_t)

    # group stats: gst = [mean, ex2] (gmask carries 1/n)
    gst = ps.tile([8, 2], F32, name=f"gst_s{s}", tag="ao", bufs=3)
    for ci in range(NC2):
        nc.tensor.matmul(gst[:], wt["gmask"][:, ci * G:(ci + 1) * G], st[ci][:],
                         start=(ci == 0), stop=(ci == NC2 - 1))
    gsb = sb.tile([8, 2], F32, name=f"gsb_s{s}", tag="gsb", bufs=2)
    nc.vector.tensor_copy(gsb[:], gst[:])
    msq = sb.tile([8, 1], F32, name=f"msq_s{s}", tag="msq", bufs=2)
    nc.vector.tensor_mul(msq[:], gsb[:, 0:1], gsb[:, 0:1])
    var = sb.tile([8, 1], F32, name=f"var_s{s}", tag="var", bufs=2)
    nc.vector.scalar_tensor_tensor(var[:], in0=gsb[:, 1:2], scalar=EPS,
                                   in1=msq[:], op0=ALU.add, op1=ALU.subtract)
    # rstd = rsqrt(var): fast-inverse-sqrt bit trick + 2 Newton steps
    ish = sb.tile([8, 1], I32, name=f"ish_s{s}", tag="ish", bufs=2)
    nc.vector.tensor_scalar(ish[:], var[:].bitcast(I32), 1, None,
                            op0=ALU.arith_shift_right)
    yib = sb.tile([8, 1], I32, name=f"yib_s{s}", tag="yib", bufs=2)
    nc.vector.tensor_tensor(yib[:], wt["magic"][0:8, :].bitcast(I32), ish[:],
                            op=ALU.subtract)
    y = yib[:].bitcast(F32)
    for it in range(2):
        ta = sb.tile([8, 1], F32, name=f"ta{it}_s{s}", tag=f"ta{it}", bufs=2)
        nc.vector.tensor_mul(ta[:], y, y)
        tb = sb.tile([8, 1], F32, name=f"tb{it}_s{s}", tag=f"tb{it}", bufs=2)
        nc.vector.tensor_mul(tb[:], ta[:], var[:])
        tcr = sb.tile([8, 1], F32, name=f"tc{it}_s{s}", tag=f"tc{it}", bufs=2)
        nc.vector.tensor_scalar(tcr[:], tb[:], -0.5, 1.5, op0=ALU.mult,
                                op1=ALU.add)
        yn = sb.tile([8, 1], F32, name=f"yn{it}_s{s}", tag=f"yn{it}", bufs=2)
        nc.vector.tensor_mul(yn[:], y, tcr[:])
        y = yn[:]
    # gv2 = [rstd, mean*rstd] feeds the gamma-scaled broadcast matmul
    gv2 = sb.tile([8, 2], F32, name=f"gv2_s{s}", tag="gv2", bufs=2)
    nc.vector.tensor_copy(gv2[:, 0:1], y)
    nc.vector.tensor_mul(gv2[:, 1:2], y, gsb[:, 0:1])

    scc, shc = [], []
    for ci in range(NC2):
        # mr = [gamma*rstd, gamma*mean*rstd] per channel
        mr = ps.tile([128, 2], F32, name=f"mr_s{s}c{ci}", tag="ao", bufs=3)
        nc.tensor.matmul(mr[:], wt["maskTg"][:, ci * 128:(ci + 1) * 128],
                         gv2[:], start=True, stop=True)
        sh_t = sb.tile([128, 1], F32, name=f"sh_s{s}c{ci}", tag=f"sh{ci}",
                       bufs=S)
        nc.vector.tensor_sub(sh_t[:], wt["beta"][ci], mr[:, 1:2])
        shc.append(sh_t)
        if s == 0:
            # sample 0: h reads the scale column straight from PSUM (one
            # less hop on the kernel-start critical chain); h follows
            # immediately so the mr bank is released right away
            scc.append(mr[:, 0:1])
        else:
            sc_t = sb.tile([128, 1], F32, name=f"scc_s{s}c{ci}",
                           tag=f"scc{ci}", bufs=S)
            nc.vector.tensor_copy(sc_t[:], mr[:, 0:1])
            scc.append(sc_t)
    return xt, scc, shc


def _emit_h(nc, pools, wt, s, stats):
    """Apply normalization: h = x*scc + sh (float32r). Emitted in half-width
    pieces, both chunks' half 0 first, so the first QKV matmuls (which only
    read h[:, 0:512]) unblock one DVE pass earlier."""
    sb, ps = pools
    xt, scc, shc = stats
    ht = [sb.tile([128, HW], F32R, name=f"h_s{s}c{ci}", tag=f"h{ci}", bufs=2)
          for ci in range(NC2)]
    for hh in range(NH2):
        hs = slice(hh * 512, (hh + 1) * 512)
        for ci in range(NC2):
            nc.vector.tensor_scalar(ht[ci][:, hs], xt[ci][:, hs],
                                    scc[ci], shc[ci][:],
                                    op0=ALU.mult, op1=ALU.add)
    return xt, ht


def _emit_attn(nc, pools, wt, s, xt, ht, out_ap, mid_cb=None, last=False):
    """QKV + attention + projection + residual for sample s. mid_cb is
    emitted between the two query-half blocks (used to interleave the next
    sample's GroupNorm at a priority below this sample's first half)."""
    sb, ps = pools

    # q, k in (C, HW) layout
    q_sb, k_sb = [], []
    for ci in range(NC2):
        q_t = sb.tile([128, HW], F32R, name=f"q_s{s}c{ci}", tag=f"q{ci}", bufs=3)
        k_t = sb.tile([128, HW], F32R, name=f"k_s{s}c{ci}", tag=f"k{ci}", bufs=3)
        for ih in range(NH2):
            hs = slice(ih * 512, (ih + 1) * 512)
            qp = ps.tile([128, 512], F32, name=f"qp_s{s}c{ci}h{ih}", tag="big", bufs=5)
            for cc in range(NC2):
                nc.tensor.matmul(
                    qp[:],
                    wt["wq"][cc][:, ci * 128:(ci + 1) * 128],
                    ht[cc][:, hs],
                    start=(cc == 0), stop=(cc == NC2 - 1))
            nc.scalar.add(q_t[:, hs], qp[:], wt["bq"][ci])
            kp = ps.tile([128, 512], F32, name=f"kp_s{s}c{ci}h{ih}", tag="big", bufs=5)
            for cc in range(NC2):
                nc.tensor.matmul(
                    kp[:],
                    wt["wq"][cc][:, C + ci * 128:C + (ci + 1) * 128],
                    ht[cc][:, hs],
                    start=(cc == 0), stop=(cc == NC2 - 1))
            nc.scalar.add(k_t[:, hs], kp[:], wt["bk"][ci])
        q_sb.append(q_t)
        k_sb.append(k_t)

    # vT in (HW, C) layout, 8 chunks of 128 positions; two chunks share one
    # PSUM bank (disjoint column halves; only the first matmul clears the
    # bank) so one DVE copy drains two chunks.
    v_sb = []
    for jp in range(HW // 256):
        vp = ps.tile([128, 2 * C], F32, name=f"vp_s{s}p{jp}", tag="big", bufs=5)
        for sub in range(2):
            j = 2 * jp + sub
            for cc in range(NC2):
                nc.tensor.matmul(
                    vp[:, sub * C:(sub + 1) * C],
                    ht[cc][:, j * 128:(j + 1) * 128],
                    wt["wq"][cc][:, 2 * C:3 * C],
                    start=(sub == 0 and cc == 0),
                    stop=(sub == 1 and cc == NC2 - 1))
        v_t = sb.tile([128, 2 * C], F32R, name=f"v_s{s}p{jp}", tag="vt", bufs=6)
        nc.vector.tensor_copy(v_t[:], vp[:])
        v_sb.append(v_t)

    # attention
    mid_res = None
    for ih in range(NH2):
        hs = slice(ih * 512, (ih + 1) * 512)
        pt = []
        for j in range(HW // 128):
            sp = ps.tile([128, 512], F32, name=f"sp_s{s}h{ih}j{j}", tag="big", bufs=5)
            for cc in range(NC2):
                nc.tensor.matmul(
                    sp[:],
                    k_sb[cc][:, j * 128:(j + 1) * 128],
                    q_sb[cc][:, hs],
                    start=(cc == 0), stop=(cc == NC2 - 1))
            p_t = sb.tile([128, 512], F32R, name=f"p_s{s}h{ih}j{j}", tag="pt", bufs=10)
            nc.scalar.activation(p_t[:], sp[:], ACTF.Exp)
            pt.append(p_t)
        # softmax denominators, broadcast to all 128 partitions for free by
        # an all-ones stationary operand (matmul cost is independent of M)
        dn = ps.tile([128, 512], F32, name=f"dn_s{s}h{ih}", tag="ao", bufs=3)
        for j in range(HW // 128):
            nc.tensor.matmul(dn[:], wt["ones_mat"][:], pt[j][:],
                             start=(j == 0), stop=(j == HW // 128 - 1))
        # attnout accumulation hides the denominator chain
        ao_sb = []
        for ci in range(NC2):
            ao = ps.tile([128, 512], F32, name=f"ao_s{s}h{ih}c{ci}", tag="ao", bufs=3)
            for j in range(HW // 128):
                voff = (j % 2) * C + ci * 128
                nc.tensor.matmul(ao[:], v_sb[j // 2][:, voff:voff + 128],
                                 pt[j][:],
                                 start=(j == 0), stop=(j == HW // 128 - 1))
            ao_t = sb.tile([128, 512], F32R, name=f"aot_s{s}h{ih}c{ci}",
                           tag="ao_sb", bufs=6)
            nc.scalar.add(ao_t[:], ao[:], wt["bv"][ci])
            ao_sb.append(ao_t)
        rb_t = sb.tile([128, 512], F32, name=f"rb_s{s}h{ih}", tag="rb", bufs=3)
        nc.vector.reciprocal(rb_t[:], dn[:])

        # projection + epilogue for this query half
        hs = slice(ih * 512, (ih + 1) * 512)
        for ci in range(NC2):
            pp = ps.tile([128, 512], F32, name=f"pp_s{s}c{ci}h{ih}", tag="ao", bufs=3)
            for cc in range(NC2):
                nc.tensor.matmul(
                    pp[:],
                    wt["wp"][cc][:, ci * 128:(ci + 1) * 128],
                    ao_sb[cc][:],
                    start=(cc == 0), stop=(cc == NC2 - 1))
            nq = 2 if (last and ih == NH2 - 1 and ci == NC2 - 1) else 1
            for q in range(nq):
                qs = slice(q * 512 // nq, (q + 1) * 512 // nq)
                qh = slice(ih * 512 + q * 512 // nq,
                           ih * 512 + (q + 1) * 512 // nq)
                t_t = sb.tile([128, 512 // nq], F32,
                              name=f"t_s{s}c{ci}h{ih}q{q}", tag="t", bufs=3)
                nc.vector.tensor_mul(t_t[:], pp[:, qs], rb_t[:, qs])
                o_t = sb.tile([128, 512 // nq], F32,
                              name=f"o_s{s}c{ci}h{ih}q{q}", tag="o", bufs=4)
                nc.vector.scalar_tensor_tensor(
                    o_t[:], in0=t_t[:], scalar=wt["bp"][ci],
                    in1=xt[ci][:, qh], op0=ALU.add, op1=ALU.add)
                nc.sync.dma_start(out_ap[s, ci * 128:(ci + 1) * 128, qh],
                                  o_t[:])

        if ih == 0 and mid_cb is not None:
            mid_res = mid_cb()
    return mid_res


def build_program(reps=1):
    nc = bacc.Bacc("TRN2", target_bir_lowering=False, debug=False,
                   enable_asserts=False, num_devices=N_CORES)

    x_ap = nc.dram_tensor("x", [S, C, HW], F32, kind="ExternalInput").ap()
    wq_ap = nc.dram_tensor("wqkvT", [C, 3 * C], F32R, kind="ExternalInput").ap()
    wp_ap = nc.dram_tensor("wprojT", [C, C], F32R, kind="ExternalInput").ap()
    ca_ap = nc.dram_tensor("constsA", [128, NC2 * G + 13], F32,
                           kind="ExternalInput").ap()
    gmt_ap = nc.dram_tensor("gmaskTg", [G, C], F32, kind="ExternalInput").ap()
    ones_ap = nc.dram_tensor("ones", [128, 128], F32R, kind="ExternalInput").ap()
    out_ap = nc.dram_tensor("out", [S, C, HW], F32, kind="ExternalOutput").ap()

    with tile.TileContext(nc) as tc:
        with (
            tc.tile_pool(name="wpool", bufs=1) as wp,
            tc.tile_pool(name="sb", bufs=2) as sb,
            tc.tile_pool(name="ps", bufs=2, space="PSUM") as ps,
        ):
            # stats-critical constants first in ONE small DMA
            constsA = wp.tile([128, NC2 * G + 13], F32, name="constsA",
                              tag="constsA")
            nc.sync.dma_start(constsA[:], ca_ap[:])
            gmask = constsA[:, 0:NC2 * G]
            cvec = constsA[:, NC2 * G:]

            def _cols(k):
                return [cvec[:, (k * NC2 + ci):(k * NC2 + ci + 1)]
                        for ci in range(NC2)]


            maskTg = wp.tile([G, C], F32, name="maskTg", tag="maskTg")
            ones_mat = wp.tile([128, 128], F32R, name="ones_mat", tag="ones_mat")
            wt = {
                "gmask": gmask,
                "maskTg": maskTg,
                "ones_mat": ones_mat,
                "bq": _cols(0),
                "bk": _cols(1),
                "bv": _cols(2),
                "bp": _cols(3),
                "gamma": _cols(4),
                "beta": _cols(5),
                "magic": cvec[:, 12:13],
            }

            stats = [None] * S
            stats[0] = _emit_stats(
                nc, (sb, ps), wt, 0, x_ap,
                post_x_cb=lambda: nc.sync.dma_start(maskTg[:], gmt_ap[:]))
            nc.sync.dma_start(ones_mat[:], ones_ap[:])
            stats[1] = _emit_stats(nc, (sb, ps), wt, 1, x_ap)

            # big weights after sample 0's x/stats DMAs are in flight
            wq0 = wp.tile([128, 3 * C], F32R, name="wq0", tag="wq0")
            nc.sync.dma_start(wq0[:], wq_ap[0:128, :])
            wq1 = wp.tile([128, 3 * C], F32R, name="wq1", tag="wq1")
            nc.sync.dma_start(wq1[:], wq_ap[128:256, :])
            wp0 = wp.tile([128, C], F32R, name="wp0", tag="wp0")
            nc.sync.dma_start(wp0[:], wp_ap[0:128, :])
            wp1 = wp.tile([128, C], F32R, name="wp1", tag="wp1")
            nc.sync.dma_start(wp1[:], wp_ap[128:256, :])
            wt["wq"] = [wq0, wq1]
            wt["wp"] = [wp0, wp1]
            h0 = _emit_h(nc, (sb, ps), wt, 0, stats[0])

            # global sample sequence across reps: stats(i) lands at
            # mid-attention of sample i-2, h(i) at mid-attention of i-1.
            seq = [(rep, s) for rep in range(reps) for s in range(S)]
            n_seq = len(seq)
            stats_ring = {0: stats[0], 1: stats[1]}

            hs_cur = h0
            for i in range(n_seq):
                xt, ht = hs_cur

                def mid_cb(i=i):
                    if i + 2 < n_seq:
                        stats_ring[i + 2] = _emit_stats(
                            nc, (sb, ps), wt, seq[i + 2][1], x_ap)
                    if i + 1 < n_seq:
                        return _emit_h(nc, (sb, ps), wt, seq[i + 1][1],
                                       stats_ring.pop(i + 1))
                    return None

                hs_cur = _emit_attn(nc, (sb, ps), wt, seq[i][1], xt, ht,
                                    out_ap, mid_cb=mid_cb,
                                    last=(i == n_seq - 1))

    nc.compile()
    return nc


def prep_inputs(x, gamma, beta, w_qkv, b_qkv, w_proj, b_proj):
    """Host-side prep: shard x over cores, transpose/scale weights."""
    x = np.ascontiguousarray(x, dtype=np.float32).reshape(B, C, HW)
    x_shards = x.reshape(N_CORES, S, C, HW)

    scale = np.float32(1.0 / np.sqrt(np.float32(C)))
    wqkvT = np.ascontiguousarray(np.asarray(w_qkv, np.float32).T)  # (C, 3C)
    wqkvT[:, 0:C] *= scale
    b_qkv = np.asarray(b_qkv, np.float32).copy()
    bq = (b_qkv[0:C] * scale).reshape(NC2, 128)
    bk = b_qkv[C:2 * C].reshape(NC2, 128)
    bv = b_qkv[2 * C:3 * C].reshape(NC2, 128)
    wprojT = np.ascontiguousarray(np.asarray(w_proj, np.float32).T)
    bp = np.asarray(b_proj, np.float32).reshape(NC2, 128)
    gam = np.asarray(gamma, np.float32).reshape(NC2, 128)
    bet = np.asarray(beta, np.float32).reshape(NC2, 128)
    cvec = np.zeros((128, 13), np.float32)
    for k, arr in enumerate([bq, bk, bv, bp, gam, bet]):
        for ci in range(NC2):
            cvec[:, k * NC2 + ci] = arr[ci]
    cvec[:, 12] = np.uint32(0x5F3759DF).view(np.float32)

    inv_n = np.float32(1.0 / (CG * HW))
    gam_flat = np.asarray(gamma, np.float32).reshape(C)
    gmask = np.zeros((128, NC2 * G), np.float32)
    gmaskTg = np.zeros((G, C), np.float32)
    for c in range(C):
        g = c // CG
        gmaskTg[g, c] = gam_flat[c]
        gmask[c % 128, (c // 128) * G + g] = inv_n

    shared = {
        "wqkvT": np.ascontiguousarray(wqkvT),
        "wprojT": wprojT,
        "constsA": np.ascontiguousarray(np.concatenate([gmask, cvec], axis=1)),
        "gmaskTg": gmaskTg,
        "ones": np.ones((128, 128), np.float32),
    }
    return [dict(shared, x=np.ascontiguousarray(x_shards[i]))
            for i in range(N_CORES)]


_NC_CACHE = {}


def kernel(x, gamma, beta, w_qkv, b_qkv, w_proj, b_proj):
    if "nc" not in _NC_CACHE:
        _NC_CACHE["nc"] = build_program()
    nc = _NC_CACHE["nc"]
    in_maps = prep_inputs(x, gamma, beta, w_qkv, b_qkv, w_proj, b_proj)
    res = run_bass_kernel_spmd(nc, in_maps, list(range(N_CORES)))
    out = np.stack([res.results[i]["out"] for i in range(N_CORES)])
    return out.reshape(B, C, H, W)



# revision 2
# speedup vs baseline: 1.0750x; 1.0750x over previous
"""AttentionBlock (GroupNorm + single-head attention over HW tokens + proj +
residual) as a Bass/Tile kernel for 8 Trainium2 NeuronCores.

Sharding: data-parallel over batch B=32 -> 4 samples per core; weights
replicated; no collectives. Full inputs in, full output out.

Core ideas (C=256, HW=1024 per sample):
  - All large matmuls use fp8 DoubleRow (contract 2x128 K per instruction at
    0.5 cycles/row, ~4x the f32r rate). Weights are folded on the host into
    [128, 2, N] DoubleRow layouts.
  - Scores use the algebraic fold q^T k = h^T (Wq^T Wk) h with L = Wk^T Wq
    precomputed on the host (fp8e4, upscaled 4x against subnormals and
    compensated in the exp scale), so only ONE projection (t = L^T h) needs
    a PSUM drain instead of two (q and k).
  - qkv biases fold exactly into softmax: the per-query term cancels in
    p/dn, the per-key term u = h^T(Wk^T bq) enters via the exp bias column.
    The v bias folds into b_proj on the host.
  - exp(s/16 - 2): 1/sqrt(C) lives in the activation scale, the -2 shift
    keeps fp8 in range. Six j-chunks on ACT (table exp -> e4m3), two on DVE
    via a Schraudolph int8 bit-trick writing e5m2 bytes directly (HW
    converts round-to-nearest).
  - dn via ones-lhsT DoubleRow matmuls over p (partition-axis reduction on
    the PE); 1/dn staged in SBUF; ao normalized at its drain.
  - ao and the projection run in f32r (removes two fp8 quantization stages
    on the worst-case peaked-softmax rows); residual is added via an
    identity f32r matmul into the proj PSUM; ACT drains with the b_proj
    bias and DMAs out.
  - GroupNorm stats via bn_stats/bn_aggr one-pass; rstd via Newton rsqrt
    split across DVE and Pool; h = x*sc+sh split Pool/DVE (fp8 out).
  - Software pipeline: sample i's epilogue is deferred past sample i+1's
    scores emission; stats run 2 samples ahead, x DMA 3 ahead. PSUM is
    managed as static rings (b2 x3, dn x1) within the 8 banks.
"""

import numpy as np
import ml_dtypes

import concourse.bacc as bacc
import concourse.tile as tile
import concourse.mybir as mybir
from concourse.bass_utils import run_bass_kernel_spmd

F32 = mybir.dt.float32
F32R = mybir.dt.float32r
F8E4 = mybir.dt.float8e4
F8E5 = mybir.dt.float8e5
I8 = mybir.dt.int8
I32 = mybir.dt.int32
ALU = mybir.AluOpType
ACTF = mybir.ActivationFunctionType
DR = mybir.MatmulPerfMode.DoubleRow

N_CORES = 8
B, C, H, W = 32, 256, 32, 32
HW = H * W          # 1024
S = B // N_CORES    # 4 samples per core
G = 8               # groups
CG = C // G         # 32 channels per group
EPS = 1e-5
NJ = HW // 128      # 8 j-chunks
NP = NJ // 2        # 4 j-chunk pairs

SCL = 0.0625        # 1/sqrt(C)
# Schraudolph constants for e5m2 output bytes: byte = A*s + dbias
SCH_A = (4.0 / np.log(2.0)) * SCL   # for the (unscaled) u bias column
SCH_AS = SCH_A / 4                  # for raw scores (L carries a 4x upscale)
SCH_B = 60.0 - 0.169 - 2.0 * (4.0 / np.log(2.0))
# jp PAIRS handled by DVE Schraudolph e5m2 (rest: ACT table exp, e4m3)
EXP_DVE_PAIRS = (1,)


def _emit_stats(nc, pools, wt, s, x_ap, xt=None, post_x_cb=None):
    """x DMA (unless a prefetched tile is passed) + GroupNorm stats down to
    per-channel scale/shift columns."""
    sb, ps = pools

    if xt is None:
        x_t = sb.tile([128, 2, HW], F32R, name=f"x_s{s}", tag="x", bufs=4)
        src = x_ap[s].rearrange("(t p) f -> p t f", t=2)
        if s == 0:
            nc.sync.dma_start(x_t[:, :, 0:512], src[:, :, 0:512])
            nc.sync.dma_start(x_t[:, :, 512:HW], src[:, :, 512:HW])
        else:
            nc.sync.dma_start(x_t[:], src)
    else:
        x_t = xt
    if post_x_cb is not None:
        post_x_cb()

    # per-channel stats: bn_stats halves -> bn_aggr -> [mean, E[x^2]] cols
    st = []
    for ci in range(2):
        bst = sb.tile([128, 2, 6], F32, name=f"bst_s{s}c{ci}", tag=f"bst{ci}",
                      bufs=2)
        nc.vector.bn_stats(bst[:, 0, :], x_t[:, ci, 0:512].bitcast(F32))
        nc.vector.bn_stats(bst[:, 1, :], x_t[:, ci, 512:HW].bitcast(F32))
        bag = sb.tile([128, 2], F32, name=f"bag_s{s}c{ci}", tag=f"bag{ci}",
                      bufs=2)
        nc.vector.bn_aggr(bag[:], bst[:])
        # col1 := mean*mean + var  (= E[x^2])
        nc.vector.tensor_scalar(bag[:, 1:2], bag[:, 0:1], bag[:, 0:1],
                                bag[:, 1:2], op0=ALU.mult, op1=ALU.add)
        st.append(bag)

    # group stats (gmask carries 1/CG): gst = [mean_g, E2_g]
    tiny = ps.tile([128, 2, 512], F32, name=f"tiny_s{s}", tag="b2", bufs=3)
    gst = tiny[0:8, 0, 0:2]
    for ci in range(2):
        nc.tensor.matmul(gst, wt["gmask"][:, ci * G:(ci + 1) * G], st[ci][:],
                         start=(ci == 0), stop=(ci == 1))
    gsb = sb.tile([8, 2], F32, name=f"gsb_s{s}", tag="gsb", bufs=2)
    nc.vector.tensor_copy(gsb[:], gst)
    msq = sb.tile([8, 1], F32, name=f"msq_s{s}", tag="msq", bufs=2)
    nc.gpsimd.tensor_mul(msq[:], gsb[:, 0:1], gsb[:, 0:1])
    var = sb.tile([8, 1], F32, name=f"var_s{s}", tag="var", bufs=2)
    nc.vector.scalar_tensor_tensor(var[:], in0=gsb[:, 1:2], scalar=EPS,
                                   in1=msq[:], op0=ALU.add, op1=ALU.subtract)
    # rstd = rsqrt(var): bit-trick seed + 2 Newton steps (all DVE, tiny)
    ish = sb.tile([8, 1], I32, name=f"ish_s{s}", tag="ish", bufs=2)
    nc.vector.tensor_scalar(ish[:], var[:].bitcast(I32), 1, None,
                            op0=ALU.arith_shift_right)
    yib = sb.tile([8, 1], I32, name=f"yib_s{s}", tag="yib", bufs=2)
    nc.gpsimd.tensor_tensor(yib[:], wt["magic"][0:8, :].bitcast(I32),
                            ish[:], op=ALU.subtract)
    y = yib[:].bitcast(F32)
    for it in range(2):
        ta = sb.tile([8, 1], F32, name=f"ta{it}_s{s}", tag=f"ta{it}", bufs=2)
        nc.gpsimd.tensor_mul(ta[:], y, y)
        tb = sb.tile([8, 1], F32, name=f"tb{it}_s{s}", tag=f"tb{it}", bufs=2)
        nc.gpsimd.tensor_mul(tb[:], ta[:], var[:])
        tcr = sb.tile([8, 1], F32, name=f"tc{it}_s{s}", tag=f"tc{it}", bufs=2)
        nc.gpsimd.tensor_scalar(tcr[:], tb[:], -0.5, 1.5, op0=ALU.mult,
                                op1=ALU.add)
        yn = sb.tile([8, 1], F32, name=f"yn{it}_s{s}", tag=f"yn{it}", bufs=2)
        nc.gpsimd.tensor_mul(yn[:], y, tcr[:])
        y = yn[:]
    gv2 = sb.tile([8, 2], F32, name=f"gv2_s{s}", tag="gv2", bufs=2)
    nc.gpsimd.tensor_copy(gv2[:, 0:1], y)
    nc.gpsimd.tensor_mul(gv2[:, 1:2], y, gsb[:, 0:1])

    # per-channel [gamma*rstd, gamma*mean*rstd] -> scale/shift cols in SBUF
    ssc = sb.tile([128, 2, 2], F32, name=f"ssc_s{s}", tag="ssc", bufs=3)
    for ci in range(2):
        mr = tiny[:, 0, 4 + 4 * ci:6 + 4 * ci]
        nc.tensor.matmul(mr, wt["maskTg"][:, ci * 128:(ci + 1) * 128],
                         gv2[:], start=True, stop=True)
        nc.vector.tensor_copy(ssc[:, ci, 0:1], mr[:, 0:1])
        nc.vector.tensor_sub(ssc[:, ci, 1:2], wt["beta"][:, ci, :], mr[:, 1:2])
    return x_t, ssc


def _emit_h(nc, pools, wt, s, stats):
    """h = x*sc + sh -> fp8e4. Chunk 0 runs on Pool (two TT passes with
    broadcast columns; Pool has no ptr-scalar op), chunk 1 on DVE."""
    sb, ps = pools
    x_t, ssc = stats
    h_t = sb.tile([128, 2, HW], F8E4, name=f"h_s{s}", tag="h", bufs=2)
    hm = sb.tile([128, HW], F32, name=f"hm_s{s}", tag="hm", bufs=2)
    nc.gpsimd.tensor_tensor(hm[:], x_t[:, 0, :].bitcast(F32),
                            ssc[:, 0, 0:1].to_broadcast((128, HW)),
                            op=ALU.mult)
    nc.gpsimd.tensor_tensor(h_t[:, 0, :], hm[:],
                            ssc[:, 0, 1:2].to_broadcast((128, HW)),
                            op=ALU.add)
    nc.vector.tensor_scalar(h_t[:, 1, :], x_t[:, 1, :].bitcast(F32),
                            ssc[:, 1, 0:1], ssc[:, 1, 1:2],
                            op0=ALU.mult, op1=ALU.add)
    return x_t, h_t


def _emit_qkv_t_mm(nc, pools, wt, s, h_t):
    """t = L^T h matmuls into PSUM."""
    sb, ps = pools
    t_sb = sb.tile([128, 2, HW], F8E4, name=f"t_s{s}", tag="t", bufs=2)
    tps = []
    for ci in range(2):
        tp = ps.tile([128, 2, 512], F32, name=f"tp_s{s}c{ci}", tag="b2",
                     bufs=3)
        for ih in range(2):
            hs = slice(ih * 512, (ih + 1) * 512)
            nc.tensor.matmul(tp[:, ih, :],
                             wt["L"][:, :, ci * 128:(ci + 1) * 128],
                             h_t[:, :, hs], start=True, stop=True,
                             perf_mode=DR)
        tps.append(tp)
    return t_sb, tps


def _emit_qkv_t_drain(nc, pools, wt, t_sb, tps):
    """t PSUM -> fp8 SBUF on ACT (queued right after this cycle's exp)."""
    for ci in range(2):
        nc.scalar.copy(t_sb[:, ci, :],
                       tps[ci][:].rearrange("p a b -> p (a b)"))


def _emit_qkv_v(nc, pools, wt, s, h_t):
    """vT (fp8e4, (HW,C) chunk-pair layout); drain on DVE (off critical
    path, emitted late)."""
    sb, ps = pools
    v_sb = []
    for vp2 in range(2):
        vp = ps.tile([128, 2, 512], F32, name=f"vp_s{s}p{vp2}", tag="b2",
                     bufs=3)
        vpv = vp[:].rearrange("p a (c d) -> p (a c) d", c=2)
        for sub in range(4):
            j = vp2 * 4 + sub
            nc.tensor.matmul(vpv[:, sub, :],
                             h_t[:, :, j * 128:(j + 1) * 128],
                             wt["Wv"][:], start=True, stop=True, perf_mode=DR)
        v_t = sb.tile([128, 4, 256], F8E4, name=f"v_s{s}p{vp2}",
                      tag=f"v{vp2}", bufs=2)
        nc.vector.tensor_copy(v_t[:], vpv)
        v_sb.append(v_t)
    return v_sb


def _emit_ubias(nc, pools, wt, s, h_t):
    """u[j] = h^T (Wk^T bq) tiny matmuls + the two exp bias column sets."""
    sb, ps = pools
    up = ps.tile([128, 2, 512], F32, name=f"up_s{s}", tag="b2", bufs=3)
    for j in range(NJ):
        nc.tensor.matmul(up[:, 0, j:j + 1],
                         h_t[:, :, j * 128:(j + 1) * 128],
                         wt["wkbq"][:], start=True, stop=True, perf_mode=DR)
    ebias = sb.tile([128, 2 * NJ], F32, name=f"eb_s{s}", tag="eb", bufs=2)
    nc.vector.tensor_scalar(ebias[:, 0:NJ], up[:, 0, 0:NJ], SCL, -2.0,
                            op0=ALU.mult, op1=ALU.add)
    nc.vector.tensor_scalar(ebias[:, NJ:2 * NJ], up[:, 0, 0:NJ], SCH_A,
                            SCH_B, op0=ALU.mult, op1=ALU.add)
    p_sb = [sb.tile([128, 2, HW],
                    F8E5 if jp in EXP_DVE_PAIRS else F8E4,
                    name=f"p_s{s}j{jp}", tag=f"p{jp}", bufs=2)
            for jp in range(NP)]
    return ebias, p_sb


def _emit_scores_j(nc, pools, wt, s, h_t, t_sb, ebias, p_sb, js):
    """Scores + exp for j-chunks in js."""
    sb, ps = pools
    for j in js:
        sp = ps.tile([128, 2, 512], F32, name=f"sp_s{s}j{j}", tag="b2",
                     bufs=3)
        for ih in range(2):
            hs = slice(ih * 512, (ih + 1) * 512)
            nc.tensor.matmul(sp[:, ih, :],
                             t_sb[:, :, j * 128:(j + 1) * 128],
                             h_t[:, :, hs], start=True, stop=True,
                             perf_mode=DR)
        spf = sp[:].rearrange("p a b -> p (a b)")
        pdst = p_sb[j // 2][:, j % 2, :]
        if j // 2 in EXP_DVE_PAIRS:
            nc.vector.tensor_scalar(pdst.bitcast(I8), spf, SCH_AS,
                                    ebias[:, NJ + j:NJ + j + 1],
                                    op0=ALU.mult, op1=ALU.add)
        else:
            nc.scalar.activation(pdst, spf, ACTF.Exp,
                                 bias=ebias[:, j:j + 1], scale=wt["sclcol"])


def _emit_dn(nc, pools, wt, s, p_sb):
    sb, ps = pools
    dn = ps.tile([128, 2, 512], F32, name=f"dn_s{s}", tag="dn", bufs=1)
    for ih in range(2):
        hs = slice(ih * 512, (ih + 1) * 512)
        for jp in range(NP):
            nc.tensor.matmul(dn[:, ih, :],
                             wt["ones5" if jp in EXP_DVE_PAIRS
                                else "ones4"][:],
                             p_sb[jp][:, :, hs],
                             start=(jp == 0), stop=(jp == NP - 1),
                             perf_mode=DR)
    # HW allows only one PSUM operand per DVE op: stage dn in SBUF for the
    # divides (also releases the dn bank early)
    rb = sb.tile([128, 2, 512], F32, name=f"rb_s{s}", tag="rb", bufs=2)
    nc.vector.reciprocal(rb[:], dn[:])
    return rb


def _emit_out_ih(nc, pools, wt, s, ih, x_t, p_sb, v_sb, dn, out_ap):
    """One query-half: ao (/dn) -> proj + residual + bias -> DMA."""
    sb, ps = pools
    hs = slice(ih * 512, (ih + 1) * 512)
    ao = ps.tile([128, 2, 512], F32, name=f"ao_s{s}h{ih}", tag="b2",
                 bufs=3)
    for ci in range(2):
        for jp in range(NP):
            nc.tensor.matmul(
                ao[:, ci, :],
                v_sb[jp // 2][:, 2 * (jp % 2):2 * (jp % 2) + 2,
                              ci * 128:(ci + 1) * 128],
                p_sb[jp][:, :, hs],
                start=(jp == 0), stop=(jp == NP - 1), perf_mode=DR)
    ao_sb = sb.tile([128, 2, 512], F32R, name=f"aos_s{s}h{ih}",
                    tag="aos", bufs=3)
    for ci in range(2):
        nc.vector.tensor_mul(ao_sb[:, ci, :], ao[:, ci, :], dn[:, ih, :])

    pp = ps.tile([128, 2, 512], F32, name=f"pp_s{s}h{ih}", tag="b2",
                 bufs=3)
    for ci in range(2):
        for cc in range(2):
            nc.tensor.matmul(pp[:, ci, :],
                             wt["Wp"][:, cc, ci * 128:(ci + 1) * 128],
                             ao_sb[:, cc, :], start=(cc == 0), stop=False)
        nc.tensor.matmul(pp[:, ci, :], wt["I128"][:],
                         x_t[:, ci, hs],
                         start=False, stop=True)
    o_sb = sb.tile([128, 2, 512], F32, name=f"o_s{s}h{ih}",
                   tag=f"o{ih}", bufs=2)
    for ci in range(2):
        nc.scalar.add(o_sb[:, ci, :], pp[:, ci, :], wt["bp"][:, ci, :])
    nc.sync.dma_start(
        out_ap[s, :, hs].rearrange("(t p) f -> p t f", t=2), o_sb[:])


def build_program(reps=1):
    nc = bacc.Bacc("TRN2", target_bir_lowering=False, debug=False,
                   enable_asserts=False, num_devices=N_CORES)

    x_ap = nc.dram_tensor("x", [S, C, HW], F32R, kind="ExternalInput").ap()
    L_ap = nc.dram_tensor("L", [128, 2, C], F8E4, kind="ExternalInput").ap()
    wv_ap = nc.dram_tensor("Wv", [128, 2, C], F8E4, kind="ExternalInput").ap()
    wp_ap = nc.dram_tensor("Wp", [128, 2, C], F32R, kind="ExternalInput").ap()
    wkbq_ap = nc.dram_tensor("wkbq", [128, 2, 1], F8E4,
                             kind="ExternalInput").ap()
    o4_ap = nc.dram_tensor("ones4", [128, 2, 128], F8E4,
                           kind="ExternalInput").ap()
    o5_ap = nc.dram_tensor("ones5", [128, 2, 128], F8E5,
                           kind="ExternalInput").ap()
    eye_ap = nc.dram_tensor("eye", [128, 128], F32R, kind="ExternalInput").ap()
    ca_ap = nc.dram_tensor("constsA", [128, 2 * G + 8], F32,
                           kind="ExternalInput").ap()
    gmt_ap = nc.dram_tensor("gmaskTg", [G, C], F32, kind="ExternalInput").ap()
    out_ap = nc.dram_tensor("out", [S, C, HW], F32, kind="ExternalOutput").ap()

    with tile.TileContext(nc) as tc:
        with (
            tc.tile_pool(name="wpool", bufs=1) as wp,
            tc.tile_pool(name="sb", bufs=2) as sb,
            tc.tile_pool(name="ps", bufs=2, space="PSUM") as ps,
        ):
            constsA = wp.tile([128, 2 * G + 8], F32, name="constsA",
                              tag="constsA")
            nc.sync.dma_start(constsA[:], ca_ap[:])
            maskTg = wp.tile([G, C], F32, name="maskTg", tag="maskTg")
            eye = wp.tile([128, 128], F32R, name="eye", tag="eye")
            ones4 = wp.tile([128, 2, 128], F8E4, name="ones4", tag="ones4")
            ones5 = wp.tile([128, 2, 128], F8E5, name="ones5", tag="ones5")
            L_t = wp.tile([128, 2, C], F8E4, name="L", tag="L")
            wv_t = wp.tile([128, 2, C], F8E4, name="Wv", tag="Wv")
            wp_t = wp.tile([128, 2, C], F32R, name="Wp", tag="Wp")
            wkbq = wp.tile([128, 2, 1], F8E4, name="wkbq", tag="wkbq")

            wt = {
                "gmask": constsA[:, 0:2 * G],
                "maskTg": maskTg,
                "magic": constsA[:, 2 * G:2 * G + 1],
                "sclcol": constsA[:, 2 * G + 1:2 * G + 2],
                "s16col": constsA[:, 2 * G + 6:2 * G + 7],
                "beta": constsA[:, 2 * G + 2:2 * G + 4]
                        .rearrange("p (t o) -> p t o", t=2),
                "bp": constsA[:, 2 * G + 4:2 * G + 6]
                        .rearrange("p (t o) -> p t o", t=2),
                "L": L_t, "Wv": wv_t, "Wp": wp_t, "wkbq": wkbq,
                "ones4": ones4, "ones5": ones5, "I128": eye,
            }

            pools = (sb, ps)
            stats = {}
            stats[0] = _emit_stats(
                nc, pools, wt, 0, x_ap,
                post_x_cb=lambda: (nc.sync.dma_start(maskTg[:], gmt_ap[:]),
                                   nc.sync.dma_start(eye[:], eye_ap[:]),
                                   nc.sync.dma_start(ones4[:], o4_ap[:]),
                                   nc.sync.dma_start(ones5[:], o5_ap[:]),
                                   nc.sync.dma_start(wkbq[:], wkbq_ap[:])))
            stats[1] = _emit_stats(nc, pools, wt, 1, x_ap)
            nc.sync.dma_start(L_t[:], L_ap[:])
            nc.sync.dma_start(wv_t[:], wv_ap[:])
            nc.sync.dma_start(wp_t[:], wp_ap[:])

            seq = [(rep, s) for rep in range(reps) for s in range(S)]
            n_seq = len(seq)
            h_cur = _emit_h(nc, pools, wt, 0, stats.pop(0))
            t_sb0, tps0 = _emit_qkv_t_mm(nc, pools, wt, 0, h_cur[1])
            _emit_qkv_t_drain(nc, pools, wt, t_sb0, tps0)
            v0 = _emit_qkv_v(nc, pools, wt, 0, h_cur[1])

            state = {"h": h_cur, "t": t_sb0, "v": v0}
            pend = None
            for i in range(n_seq):
                s = seq[i][1]
                x_t, h_t = state["h"]
                t_sb, v_sb = state["t"], state["v"]

                # 1. exp bias columns
                ebias, p_sb = _emit_ubias(nc, pools, wt, s, h_t)

                # 2. next sample's h (Pool + DVE early in their queues)
                nh = None
                if i + 1 < n_seq:
                    nh = _emit_h(nc, pools, wt, seq[i + 1][1],
                                 stats.pop(i + 1))
                    state["h"] = nh

                # 3. scores + exp j0..j7
                _emit_scores_j(nc, pools, wt, s, h_t, t_sb, ebias, p_sb,
                               range(NJ))

                # 4. previous sample's epilogue (ao/mult/pp after exp in
                # queues; its psum ring slots free as exp drains them)
                if pend is not None:
                    _emit_out_ih(nc, pools, wt, pend["s"], 0, pend["x"],
                                 pend["p"], pend["v"], pend["rb"], out_ap)
                    _emit_out_ih(nc, pools, wt, pend["s"], 1, pend["x"],
                                 pend["p"], pend["v"], pend["rb"], out_ap)
                    pend = None

                # 5. next sample's t matmuls + ACT drains (after exp_i)
                if i + 1 < n_seq:
                    nt, ntps = _emit_qkv_t_mm(nc, pools, wt, seq[i + 1][1],
                                              nh[1])
                    _emit_qkv_t_drain(nc, pools, wt, nt, ntps)
                    state["t"] = nt

                # 6. dn + reciprocal for sample i
                rb = _emit_dn(nc, pools, wt, s, p_sb)

                # 7. next sample's v
                if i + 1 < n_seq:
                    state["v"] = _emit_qkv_v(nc, pools, wt, seq[i + 1][1],
                                             nh[1])

                # 8. stats for i+2 + x DMA for i+3
                if i + 2 < n_seq:
                    stats[i + 2] = _emit_stats(
                        nc, pools, wt, seq[i + 2][1], x_ap,
                        xt=stats.pop(("xt", i + 2), None))
                if i + 3 < n_seq:
                    xs = seq[i + 3][1]
                    xt3 = sb.tile([128, 2, HW], F32R, name=f"x_s{xs}",
                                  tag="x", bufs=4)
                    nc.sync.dma_start(
                        xt3[:],
                        x_ap[xs].rearrange("(t p) f -> p t f", t=2))
                    stats[("xt", i + 3)] = xt3

                pend = {"s": s, "x": x_t, "p": p_sb, "v": v_sb, "rb": rb}

            _emit_out_ih(nc, pools, wt, pend["s"], 0, pend["x"], pend["p"],
                         pend["v"], pend["rb"], out_ap)
            _emit_out_ih(nc, pools, wt, pend["s"], 1, pend["x"], pend["p"],
                         pend["v"], pend["rb"], out_ap)

    nc.compile()
    return nc


def _f8(x, dt=ml_dtypes.float8_e4m3):
    return np.asarray(x, np.float32).astype(dt)


def _fold(mat):
    """(256, N) -> [128, 2, N] DoubleRow contraction layout."""
    n = mat.shape[1]
    return np.ascontiguousarray(mat.reshape(2, 128, n).transpose(1, 0, 2))


def prep_inputs(x, gamma, beta, w_qkv, b_qkv, w_proj, b_proj):
    x = np.ascontiguousarray(x, dtype=np.float32).reshape(B, C, HW)
    x_shards = x.reshape(N_CORES, S, C, HW)

    w_qkv = np.asarray(w_qkv, np.float32)
    b_qkv = np.asarray(b_qkv, np.float32)
    w_proj = np.asarray(w_proj, np.float32)
    b_proj = np.asarray(b_proj, np.float32)
    gamma = np.asarray(gamma, np.float32)
    beta = np.asarray(beta, np.float32)

    Wq, Wk, Wv = w_qkv[0:C], w_qkv[C:2 * C], w_qkv[2 * C:3 * C]
    L = Wk.T @ Wq                      # L[b, a]
    wkbq = Wk.T @ b_qkv[0:C]           # per-j score bias term
    bp_eff = b_proj + w_proj @ b_qkv[2 * C:3 * C]

    # constsA: gmask (1/CG for bn-path) | magic | scale | beta cols | bp cols
    gmask = np.zeros((128, 2 * G), np.float32)
    for c in range(C):
        gmask[c % 128, (c // 128) * G + c // CG] = 1.0 / CG
    cvec = np.zeros((128, 8), np.float32)
    cvec[:, 0] = np.uint32(0x5F3759DF).view(np.float32)
    cvec[:, 1] = SCL / 4
    cvec[:, 2:4] = beta.reshape(2, 128).T
    cvec[:, 4:6] = bp_eff.reshape(2, 128).T
    cvec[:, 6] = 1.0 / 16

    gmaskTg = np.zeros((G, C), np.float32)
    for c in range(C):
        gmaskTg[c // CG, c] = gamma[c]

    shared = {
        "L": _f8(_fold(L * 4.0)),
        "Wv": _f8(_fold(Wv.T * 4.0)),
        "Wp": _fold(w_proj.T).astype(np.float32),
        "wkbq": _f8(_fold(wkbq.reshape(C, 1))),
        "ones4": np.full((128, 2, 128), 4.0, ml_dtypes.float8_e4m3),
        "ones5": np.full((128, 2, 128), 4.0, ml_dtypes.float8_e5m2),
        "eye": np.eye(128, dtype=np.float32),
        "constsA": np.ascontiguousarray(np.concatenate([gmask, cvec], 1)),
        "gmaskTg": gmaskTg,
    }
    return [dict(shared, x=np.ascontiguousarray(x_shards[i]))
            for i in range(N_CORES)]


_NC_CACHE = {}


def kernel(x, gamma, beta, w_qkv, b_qkv, w_proj, b_proj):
    if "nc" not in _NC_CACHE:
        _NC_CACHE["nc"] = build_program()
    nc = _NC_CACHE["nc"]
    in_maps = prep_inputs(x, gamma, beta, w_qkv, b_qkv, w_proj, b_proj)
    res = run_bass_kernel_spmd(nc, in_maps, list(range(N_CORES)))
    out = np.stack([res.results[i]["out"] for i in range(N_CORES)])
    return out.reshape(B, C, H, W)


# revision 3
# speedup vs baseline: 1.3586x; 1.2637x over previous
"""AttentionBlock (GroupNorm + single-head attention over HW tokens + proj +
residual) as a Bass/Tile kernel for 8 Trainium2 NeuronCores.

Sharding: data-parallel over batch B=32 -> 4 samples per core; weights
replicated; no collectives. Full inputs in, full output out.

Core ideas (C=256, HW=1024 per sample):
  - All large matmuls use fp8 DoubleRow (contract 2x128 K per instruction at
    0.5 cycles/row, ~4x the f32r rate). Weights are folded on the host into
    [128, 2, N] DoubleRow layouts.
  - Scores use the algebraic fold q^T k = h^T (Wq^T Wk) h with L = Wk^T Wq
    precomputed on the host (fp8e4, upscaled 4x against subnormals and
    compensated in the exp scale), so only ONE projection (t = L^T h) needs
    a PSUM drain instead of two (q and k).
  - qkv biases fold exactly into softmax: the per-query term cancels in
    p/dn, the per-key term u = h^T(Wk^T bq) enters via the exp bias column.
    The v bias folds into b_proj on the host.
  - exp(s/16 - 2): 1/sqrt(C) lives in the activation scale, the -2 shift
    keeps fp8 in range. Six j-chunks on ACT (table exp -> e4m3), two on DVE
    via a Schraudolph int8 bit-trick writing e5m2 bytes directly (HW
    converts round-to-nearest).
  - dn via ones-lhsT DoubleRow matmuls over p (partition-axis reduction on
    the PE); 1/dn staged in SBUF; ao normalized at its drain.
  - ao and the projection run in f32r (removes two fp8 quantization stages
    on the worst-case peaked-softmax rows); residual is added via an
    identity f32r matmul into the proj PSUM; ACT drains with the b_proj
    bias and DMAs out.
  - GroupNorm stats via bn_stats/bn_aggr one-pass; rstd via Newton rsqrt
    split across DVE and Pool; h = x*sc+sh split Pool/DVE (fp8 out).
  - Software pipeline: sample i's epilogue is deferred past sample i+1's
    scores emission; stats run 2 samples ahead, x DMA 3 ahead. PSUM is
    managed as static rings (b2 x3, dn x1) within the 8 banks.
"""

import numpy as np
import ml_dtypes

import concourse.bacc as bacc
import concourse.tile as tile
import concourse.mybir as mybir
from concourse.bass_utils import run_bass_kernel_spmd

F32 = mybir.dt.float32
F32R = mybir.dt.float32r
F8E4 = mybir.dt.float8e4
F8E5 = mybir.dt.float8e5
I8 = mybir.dt.int8
I32 = mybir.dt.int32
ALU = mybir.AluOpType
ACTF = mybir.ActivationFunctionType
DR = mybir.MatmulPerfMode.DoubleRow

N_CORES = 8
B, C, H, W = 32, 256, 32, 32
HW = H * W          # 1024
S = B // N_CORES    # 4 samples per core
G = 8               # groups
CG = C // G         # 32 channels per group
EPS = 1e-5
NJ = HW // 128      # 8 j-chunks
NP = NJ // 2        # 4 j-chunk pairs

SCL = 0.0625        # 1/sqrt(C)
# Schraudolph constants for e5m2 output bytes: byte = A*s + dbias
SCH_A = (4.0 / np.log(2.0)) * SCL   # for the (unscaled) u bias column
SCH_AS = SCH_A / 4                  # for raw scores (L carries a 4x upscale)
SCH_B = 60.0 - 0.169 - 2.0 * (4.0 / np.log(2.0))
# jp PAIRS handled by DVE Schraudolph e5m2 (rest: ACT table exp, e4m3)
EXP_DVE_PAIRS = (1,)


def _emit_stats(nc, pools, wt, s, x_ap, xt=None, post_x_cb=None):
    """x DMA (unless a prefetched tile is passed) + GroupNorm stats down to
    per-channel scale/shift columns."""
    sb, ps = pools

    if xt is None:
        x_t = sb.tile([128, 2, HW], F32R, name=f"x_s{s}", tag="x", bufs=4)
        src = x_ap[s].rearrange("(t p) f -> p t f", t=2)
        if s == 0:
            nc.sync.dma_start(x_t[:, :, 0:512], src[:, :, 0:512])
            nc.sync.dma_start(x_t[:, :, 512:HW], src[:, :, 512:HW])
        else:
            nc.sync.dma_start(x_t[:], src)
    else:
        x_t = xt
    if post_x_cb is not None:
        post_x_cb()

    # per-channel stats: bn_stats halves -> bn_aggr -> [mean, E[x^2]] cols
    st = []
    for ci in range(2):
        bst = sb.tile([128, 2, 6], F32, name=f"bst_s{s}c{ci}", tag=f"bst{ci}",
                      bufs=2)
        nc.vector.bn_stats(bst[:, 0, :], x_t[:, ci, 0:512].bitcast(F32))
        nc.vector.bn_stats(bst[:, 1, :], x_t[:, ci, 512:HW].bitcast(F32))
        bag = sb.tile([128, 2], F32, name=f"bag_s{s}c{ci}", tag=f"bag{ci}",
                      bufs=2)
        nc.vector.bn_aggr(bag[:], bst[:])
        # col1 := mean*mean + var  (= E[x^2])
        nc.vector.tensor_scalar(bag[:, 1:2], bag[:, 0:1], bag[:, 0:1],
                                bag[:, 1:2], op0=ALU.mult, op1=ALU.add)
        st.append(bag)

    # group stats (gmask carries 1/CG): gst = [mean_g, E2_g]
    tiny = ps.tile([128, 2, 512], F32, name=f"tiny_s{s}", tag="b2", bufs=3)
    gst = tiny[0:8, 0, 0:2]
    for ci in range(2):
        nc.tensor.matmul(gst, wt["gmask"][:, ci * G:(ci + 1) * G], st[ci][:],
                         start=(ci == 0), stop=(ci == 1))
    gsb = sb.tile([8, 2], F32, name=f"gsb_s{s}", tag="gsb", bufs=2)
    nc.vector.tensor_copy(gsb[:], gst)
    msq = sb.tile([8, 1], F32, name=f"msq_s{s}", tag="msq", bufs=2)
    nc.gpsimd.tensor_mul(msq[:], gsb[:, 0:1], gsb[:, 0:1])
    var = sb.tile([8, 1], F32, name=f"var_s{s}", tag="var", bufs=2)
    nc.vector.scalar_tensor_tensor(var[:], in0=gsb[:, 1:2], scalar=EPS,
                                   in1=msq[:], op0=ALU.add, op1=ALU.subtract)
    # rstd = rsqrt(var): bit-trick seed + 2 Newton steps (all DVE, tiny)
    ish = sb.tile([8, 1], I32, name=f"ish_s{s}", tag="ish", bufs=2)
    nc.vector.tensor_scalar(ish[:], var[:].bitcast(I32), 1, None,
                            op0=ALU.arith_shift_right)
    yib = sb.tile([8, 1], I32, name=f"yib_s{s}", tag="yib", bufs=2)
    nc.gpsimd.tensor_tensor(yib[:], wt["magic"][0:8, :].bitcast(I32),
                            ish[:], op=ALU.subtract)
    y = yib[:].bitcast(F32)
    for it in range(2):
        ta = sb.tile([8, 1], F32, name=f"ta{it}_s{s}", tag=f"ta{it}", bufs=2)
        nc.gpsimd.tensor_mul(ta[:], y, y)
        tb = sb.tile([8, 1], F32, name=f"tb{it}_s{s}", tag=f"tb{it}", bufs=2)
        nc.gpsimd.tensor_mul(tb[:], ta[:], var[:])
        tcr = sb.tile([8, 1], F32, name=f"tc{it}_s{s}", tag=f"tc{it}", bufs=2)
        nc.gpsimd.tensor_scalar(tcr[:], tb[:], -0.5, 1.5, op0=ALU.mult,
                                op1=ALU.add)
        yn = sb.tile([8, 1], F32, name=f"yn{it}_s{s}", tag=f"yn{it}", bufs=2)
        nc.gpsimd.tensor_mul(yn[:], y, tcr[:])
        y = yn[:]
    gv2 = sb.tile([8, 2], F32, name=f"gv2_s{s}", tag="gv2", bufs=2)
    nc.gpsimd.tensor_copy(gv2[:, 0:1], y)
    nc.gpsimd.tensor_mul(gv2[:, 1:2], y, gsb[:, 0:1])

    # per-channel [gamma*rstd, gamma*mean*rstd] -> scale/shift cols in SBUF
    ssc = sb.tile([128, 2, 2], F32, name=f"ssc_s{s}", tag="ssc", bufs=3)
    for ci in range(2):
        mr = tiny[:, 0, 4 + 4 * ci:6 + 4 * ci]
        nc.tensor.matmul(mr, wt["maskTg"][:, ci * 128:(ci + 1) * 128],
                         gv2[:], start=True, stop=True)
        nc.vector.tensor_copy(ssc[:, ci, 0:1], mr[:, 0:1])
        nc.vector.tensor_sub(ssc[:, ci, 1:2], wt["beta"][:, ci, :], mr[:, 1:2])
    return x_t, ssc


def _emit_h(nc, pools, wt, s, stats):
    """h = x*sc + sh -> fp8e4. Chunk 0 runs on Pool (two TT passes with
    broadcast columns; Pool has no ptr-scalar op), chunk 1 on DVE."""
    sb, ps = pools
    x_t, ssc = stats
    h_t = sb.tile([128, 2, HW], F8E4, name=f"h_s{s}", tag="h", bufs=2)
    hm = sb.tile([128, HW], F32, name=f"hm_s{s}", tag="hm", bufs=2)
    nc.gpsimd.tensor_tensor(hm[:], x_t[:, 0, :].bitcast(F32),
                            ssc[:, 0, 0:1].to_broadcast((128, HW)),
                            op=ALU.mult)
    nc.gpsimd.tensor_tensor(h_t[:, 0, :], hm[:],
                            ssc[:, 0, 1:2].to_broadcast((128, HW)),
                            op=ALU.add)
    nc.vector.tensor_scalar(h_t[:, 1, :], x_t[:, 1, :].bitcast(F32),
                            ssc[:, 1, 0:1], ssc[:, 1, 1:2],
                            op0=ALU.mult, op1=ALU.add)
    return x_t, h_t


def _emit_qkv_t_mm(nc, pools, wt, s, h_t):
    """t = L^T h matmuls into PSUM."""
    sb, ps = pools
    t_sb = sb.tile([128, 2, HW], F8E4, name=f"t_s{s}", tag="t", bufs=2)
    tps = []
    for ci in range(2):
        tp = ps.tile([128, 2, 512], F32, name=f"tp_s{s}c{ci}", tag="b2",
                     bufs=3)
        for ih in range(2):
            hs = slice(ih * 512, (ih + 1) * 512)
            nc.tensor.matmul(tp[:, ih, :],
                             wt["L"][:, :, ci * 128:(ci + 1) * 128],
                             h_t[:, :, hs], start=True, stop=True,
                             perf_mode=DR)
        tps.append(tp)
    return t_sb, tps


def _emit_qkv_t_drain(nc, pools, wt, t_sb, tps):
    """t PSUM -> fp8 SBUF on ACT (queued right after this cycle's exp)."""
    for ci in range(2):
        nc.scalar.copy(t_sb[:, ci, :],
                       tps[ci][:].rearrange("p a b -> p (a b)"))


def _emit_qkv_v(nc, pools, wt, s, h_t):
    """vT (fp8e4, (HW,C) chunk-pair layout); drain on DVE (off critical
    path, emitted late)."""
    sb, ps = pools
    v_sb = []
    for vp2 in range(2):
        vp = ps.tile([128, 2, 512], F32, name=f"vp_s{s}p{vp2}", tag="b2",
                     bufs=3)
        vpv = vp[:].rearrange("p a (c d) -> p (a c) d", c=2)
        for sub in range(4):
            j = vp2 * 4 + sub
            nc.tensor.matmul(vpv[:, sub, :],
                             h_t[:, :, j * 128:(j + 1) * 128],
                             wt["Wv"][:], start=True, stop=True, perf_mode=DR)
        v_t = sb.tile([128, 4, 256], F8E4, name=f"v_s{s}p{vp2}",
                      tag=f"v{vp2}", bufs=2)
        if vp2 == 0:
            nc.scalar.copy(v_t[:], vpv)
        else:
            nc.vector.tensor_copy(v_t[:], vpv)
        v_sb.append(v_t)
    return v_sb


def _emit_ubias(nc, pools, wt, s, h_t):
    """u[j] = h^T (Wk^T bq) tiny matmuls + the two exp bias column sets."""
    sb, ps = pools
    up = ps.tile([128, 2, 512], F32, name=f"up_s{s}", tag="b2", bufs=3)
    for j in range(NJ):
        nc.tensor.matmul(up[:, 0, j:j + 1],
                         h_t[:, :, j * 128:(j + 1) * 128],
                         wt["wkbq"][:], start=True, stop=True, perf_mode=DR)
    ebias = sb.tile([128, 2 * NJ], F32, name=f"eb_s{s}", tag="eb", bufs=2)
    nc.vector.tensor_scalar(ebias[:, 0:NJ], up[:, 0, 0:NJ], SCL, -2.0,
                            op0=ALU.mult, op1=ALU.add)
    nc.vector.tensor_scalar(ebias[:, NJ:2 * NJ], up[:, 0, 0:NJ], SCH_A,
                            SCH_B, op0=ALU.mult, op1=ALU.add)
    p_sb = [sb.tile([128, 2, HW],
                    F8E5 if jp in EXP_DVE_PAIRS else F8E4,
                    name=f"p_s{s}j{jp}", tag=f"p{jp}", bufs=2)
            for jp in range(NP)]
    return ebias, p_sb


def _emit_scores_j(nc, pools, wt, s, h_t, t_sb, ebias, p_sb, js):
    """Scores + exp for j-chunks in js."""
    sb, ps = pools
    for j in js:
        sp = ps.tile([128, 2, 512], F32, name=f"sp_s{s}j{j}", tag="b2",
                     bufs=3)
        for ih in range(2):
            hs = slice(ih * 512, (ih + 1) * 512)
            nc.tensor.matmul(sp[:, ih, :],
                             t_sb[:, :, j * 128:(j + 1) * 128],
                             h_t[:, :, hs], start=True, stop=True,
                             perf_mode=DR)
        spf = sp[:].rearrange("p a b -> p (a b)")
        pdst = p_sb[j // 2][:, j % 2, :]
        if j // 2 in EXP_DVE_PAIRS:
            nc.vector.tensor_scalar(pdst.bitcast(I8), spf, SCH_AS,
                                    ebias[:, NJ + j:NJ + j + 1],
                                    op0=ALU.mult, op1=ALU.add)
        else:
            nc.scalar.activation(pdst, spf, ACTF.Exp,
                                 bias=ebias[:, j:j + 1], scale=wt["sclcol"])


def _emit_dn(nc, pools, wt, s, p_sb):
    sb, ps = pools
    dn = ps.tile([128, 2, 512], F32, name=f"dn_s{s}", tag="dn", bufs=1)
    for ih in range(2):
        hs = slice(ih * 512, (ih + 1) * 512)
        for jp in range(NP):
            nc.tensor.matmul(dn[:, ih, :],
                             wt["ones5" if jp in EXP_DVE_PAIRS
                                else "ones4"][:],
                             p_sb[jp][:, :, hs],
                             start=(jp == 0), stop=(jp == NP - 1),
                             perf_mode=DR)
    # HW allows only one PSUM operand per DVE op: stage dn in SBUF for the
    # divides (also releases the dn bank early)
    rb = sb.tile([128, 2, 512], F32, name=f"rb_s{s}", tag="rb", bufs=2)
    nc.vector.reciprocal(rb[:], dn[:])
    return rb


def _emit_out_ih(nc, pools, wt, s, ih, x_t, p_sb, v_sb, dn, out_ap):
    """One query-half: ao (/dn) -> proj + residual + bias -> DMA."""
    sb, ps = pools
    hs = slice(ih * 512, (ih + 1) * 512)
    ao = ps.tile([128, 2, 512], F32, name=f"ao_s{s}h{ih}", tag="b2",
                 bufs=3)
    for ci in range(2):
        for jp in range(NP):
            nc.tensor.matmul(
                ao[:, ci, :],
                v_sb[jp // 2][:, 2 * (jp % 2):2 * (jp % 2) + 2,
                              ci * 128:(ci + 1) * 128],
                p_sb[jp][:, :, hs],
                start=(jp == 0), stop=(jp == NP - 1), perf_mode=DR)
    ao_sb = sb.tile([128, 2, 512], F32R, name=f"aos_s{s}h{ih}",
                    tag="aos", bufs=3)
    for ci in range(2):
        nc.vector.tensor_mul(ao_sb[:, ci, :], ao[:, ci, :], dn[:, ih, :])

    pp = ps.tile([128, 2, 512], F32, name=f"pp_s{s}h{ih}", tag="b2",
                 bufs=3)
    for ci in range(2):
        for cc in range(2):
            nc.tensor.matmul(pp[:, ci, :],
                             wt["Wp"][:, cc, ci * 128:(ci + 1) * 128],
                             ao_sb[:, cc, :], start=(cc == 0), stop=False)
        nc.tensor.matmul(pp[:, ci, :], wt["I128"][:],
                         x_t[:, ci, hs],
                         start=False, stop=True)
    o_sb = sb.tile([128, 2, 512], F32, name=f"o_s{s}h{ih}",
                   tag=f"o{ih}", bufs=2)
    for ci in range(2):
        nc.scalar.add(o_sb[:, ci, :], pp[:, ci, :], wt["bp"][:, ci, :])
    nc.sync.dma_start(
        out_ap[s, :, hs].rearrange("(t p) f -> p t f", t=2), o_sb[:])


def build_program(reps=1):
    nc = bacc.Bacc("TRN2", target_bir_lowering=False, debug=False,
                   enable_asserts=False, num_devices=N_CORES)

    x_ap = nc.dram_tensor("x", [S, C, HW], F32R, kind="ExternalInput").ap()
    L_ap = nc.dram_tensor("L", [128, 2, C], F8E4, kind="ExternalInput").ap()
    wv_ap = nc.dram_tensor("Wv", [128, 2, C], F8E4, kind="ExternalInput").ap()
    wp_ap = nc.dram_tensor("Wp", [128, 2, C], F32R, kind="ExternalInput").ap()
    wkbq_ap = nc.dram_tensor("wkbq", [128, 2, 1], F8E4,
                             kind="ExternalInput").ap()
    o4_ap = nc.dram_tensor("ones4", [128, 2, 128], F8E4,
                           kind="ExternalInput").ap()
    o5_ap = nc.dram_tensor("ones5", [128, 2, 128], F8E5,
                           kind="ExternalInput").ap()
    eye_ap = nc.dram_tensor("eye", [128, 128], F32R, kind="ExternalInput").ap()
    ca_ap = nc.dram_tensor("constsA", [128, 2 * G + 8], F32,
                           kind="ExternalInput").ap()
    gmt_ap = nc.dram_tensor("gmaskTg", [G, C], F32, kind="ExternalInput").ap()
    out_ap = nc.dram_tensor("out", [S, C, HW], F32, kind="ExternalOutput").ap()

    with tile.TileContext(nc) as tc:
        with (
            tc.tile_pool(name="wpool", bufs=1) as wp,
            tc.tile_pool(name="sb", bufs=2) as sb,
            tc.tile_pool(name="ps", bufs=2, space="PSUM") as ps,
        ):
            constsA = wp.tile([128, 2 * G + 8], F32, name="constsA",
                              tag="constsA")
            nc.sync.dma_start(constsA[:], ca_ap[:])
            maskTg = wp.tile([G, C], F32, name="maskTg", tag="maskTg")
            eye = wp.tile([128, 128], F32R, name="eye", tag="eye")
            ones4 = wp.tile([128, 2, 128], F8E4, name="ones4", tag="ones4")
            ones5 = wp.tile([128, 2, 128], F8E5, name="ones5", tag="ones5")
            L_t = wp.tile([128, 2, C], F8E4, name="L", tag="L")
            wv_t = wp.tile([128, 2, C], F8E4, name="Wv", tag="Wv")
            wp_t = wp.tile([128, 2, C], F32R, name="Wp", tag="Wp")
            wkbq = wp.tile([128, 2, 1], F8E4, name="wkbq", tag="wkbq")

            wt = {
                "gmask": constsA[:, 0:2 * G],
                "maskTg": maskTg,
                "magic": constsA[:, 2 * G:2 * G + 1],
                "sclcol": constsA[:, 2 * G + 1:2 * G + 2],
                "s16col": constsA[:, 2 * G + 6:2 * G + 7],
                "beta": constsA[:, 2 * G + 2:2 * G + 4]
                        .rearrange("p (t o) -> p t o", t=2),
                "bp": constsA[:, 2 * G + 4:2 * G + 6]
                        .rearrange("p (t o) -> p t o", t=2),
                "L": L_t, "Wv": wv_t, "Wp": wp_t, "wkbq": wkbq,
                "ones4": ones4, "ones5": ones5, "I128": eye,
            }

            pools = (sb, ps)
            stats = {}
            stats[0] = _emit_stats(
                nc, pools, wt, 0, x_ap,
                post_x_cb=lambda: (nc.sync.dma_start(maskTg[:], gmt_ap[:]),
                                   nc.sync.dma_start(eye[:], eye_ap[:]),
                                   nc.sync.dma_start(ones4[:], o4_ap[:]),
                                   nc.sync.dma_start(ones5[:], o5_ap[:]),
                                   nc.sync.dma_start(wkbq[:], wkbq_ap[:])))
            stats[1] = _emit_stats(nc, pools, wt, 1, x_ap)
            nc.sync.dma_start(L_t[:], L_ap[:])
            nc.sync.dma_start(wv_t[:], wv_ap[:])
            nc.sync.dma_start(wp_t[:], wp_ap[:])

            seq = [(rep, s) for rep in range(reps) for s in range(S)]
            n_seq = len(seq)
            h_cur = _emit_h(nc, pools, wt, 0, stats.pop(0))
            t_sb0, tps0 = _emit_qkv_t_mm(nc, pools, wt, 0, h_cur[1])
            _emit_qkv_t_drain(nc, pools, wt, t_sb0, tps0)
            v0 = _emit_qkv_v(nc, pools, wt, 0, h_cur[1])

            state = {"h": h_cur, "t": t_sb0, "v": v0}
            pend = None
            for i in range(n_seq):
                s = seq[i][1]
                x_t, h_t = state["h"]
                t_sb, v_sb = state["t"], state["v"]

                # 1. exp bias columns
                ebias, p_sb = _emit_ubias(nc, pools, wt, s, h_t)

                # 2. next sample's h (Pool + DVE early in their queues)
                nh = None
                if i + 1 < n_seq:
                    nh = _emit_h(nc, pools, wt, seq[i + 1][1],
                                 stats.pop(i + 1))
                    state["h"] = nh

                # 3. scores + exp j0..j7
                _emit_scores_j(nc, pools, wt, s, h_t, t_sb, ebias, p_sb,
                               range(NJ))

                # 4. previous sample's epilogue (ao/mult/pp after exp in
                # queues; its psum ring slots free as exp drains them)
                if pend is not None:
                    _emit_out_ih(nc, pools, wt, pend["s"], 0, pend["x"],
                                 pend["p"], pend["v"], pend["rb"], out_ap)
                    _emit_out_ih(nc, pools, wt, pend["s"], 1, pend["x"],
                                 pend["p"], pend["v"], pend["rb"], out_ap)
                    pend = None

                # 5. next sample's t matmuls + ACT drains (after exp_i)
                if i + 1 < n_seq:
                    nt, ntps = _emit_qkv_t_mm(nc, pools, wt, seq[i + 1][1],
                                              nh[1])
                    _emit_qkv_t_drain(nc, pools, wt, nt, ntps)
                    state["t"] = nt

                # 6. dn + reciprocal for sample i
                rb = _emit_dn(nc, pools, wt, s, p_sb)

                # 7. next sample's v
                if i + 1 < n_seq:
                    state["v"] = _emit_qkv_v(nc, pools, wt, seq[i + 1][1],
                                             nh[1])

                # 8. stats for i+2 + x DMA for i+3
                if i + 2 < n_seq:
                    stats[i + 2] = _emit_stats(
                        nc, pools, wt, seq[i + 2][1], x_ap,
                        xt=stats.pop(("xt", i + 2), None))
                if i + 3 < n_seq:
                    xs = seq[i + 3][1]
                    xt3 = sb.tile([128, 2, HW], F32R, name=f"x_s{xs}",
                                  tag="x", bufs=4)
                    nc.sync.dma_start(
                        xt3[:],
                        x_ap[xs].rearrange("(t p) f -> p t f", t=2))
                    stats[("xt", i + 3)] = xt3

                pend = {"s": s, "x": x_t, "p": p_sb, "v": v_sb, "rb": rb}

            _emit_out_ih(nc, pools, wt, pend["s"], 0, pend["x"], pend["p"],
                         pend["v"], pend["rb"], out_ap)
            _emit_out_ih(nc, pools, wt, pend["s"], 1, pend["x"], pend["p"],
                         pend["v"], pend["rb"], out_ap)

    nc.compile()
    return nc


def _f8(x, dt=ml_dtypes.float8_e4m3):
    return np.asarray(x, np.float32).astype(dt)


def _fold(mat):
    """(256, N) -> [128, 2, N] DoubleRow contraction layout."""
    n = mat.shape[1]
    return np.ascontiguousarray(mat.reshape(2, 128, n).transpose(1, 0, 2))


def prep_inputs(x, gamma, beta, w_qkv, b_qkv, w_proj, b_proj):
    x = np.ascontiguousarray(x, dtype=np.float32).reshape(B, C, HW)
    x_shards = x.reshape(N_CORES, S, C, HW)

    w_qkv = np.asarray(w_qkv, np.float32)
    b_qkv = np.asarray(b_qkv, np.float32)
    w_proj = np.asarray(w_proj, np.float32)
    b_proj = np.asarray(b_proj, np.float32)
    gamma = np.asarray(gamma, np.float32)
    beta = np.asarray(beta, np.float32)

    Wq, Wk, Wv = w_qkv[0:C], w_qkv[C:2 * C], w_qkv[2 * C:3 * C]
    L = Wk.T @ Wq                      # L[b, a]
    wkbq = Wk.T @ b_qkv[0:C]           # per-j score bias term
    bp_eff = b_proj + w_proj @ b_qkv[2 * C:3 * C]

    # constsA: gmask (1/CG for bn-path) | magic | scale | beta cols | bp cols
    gmask = np.zeros((128, 2 * G), np.float32)
    for c in range(C):
        gmask[c % 128, (c // 128) * G + c // CG] = 1.0 / CG
    cvec = np.zeros((128, 8), np.float32)
    cvec[:, 0] = np.uint32(0x5F3759DF).view(np.float32)
    cvec[:, 1] = SCL / 4
    cvec[:, 2:4] = beta.reshape(2, 128).T
    cvec[:, 4:6] = bp_eff.reshape(2, 128).T
    cvec[:, 6] = 1.0 / 16

    gmaskTg = np.zeros((G, C), np.float32)
    for c in range(C):
        gmaskTg[c // CG, c] = gamma[c]

    shared = {
        "L": _f8(_fold(L * 4.0)),
        "Wv": _f8(_fold(Wv.T * 4.0)),
        "Wp": _fold(w_proj.T).astype(np.float32),
        "wkbq": _f8(_fold(wkbq.reshape(C, 1))),
        "ones4": np.full((128, 2, 128), 4.0, ml_dtypes.float8_e4m3),
        "ones5": np.full((128, 2, 128), 4.0, ml_dtypes.float8_e5m2),
        "eye": np.eye(128, dtype=np.float32),
        "constsA": np.ascontiguousarray(np.concatenate([gmask, cvec], 1)),
        "gmaskTg": gmaskTg,
    }
    return [dict(shared, x=np.ascontiguousarray(x_shards[i]))
            for i in range(N_CORES)]


_NC_CACHE = {}


def kernel(x, gamma, beta, w_qkv, b_qkv, w_proj, b_proj):
    if "nc" not in _NC_CACHE:
        _NC_CACHE["nc"] = build_program()
    nc = _NC_CACHE["nc"]
    in_maps = prep_inputs(x, gamma, beta, w_qkv, b_qkv, w_proj, b_proj)
    res = run_bass_kernel_spmd(nc, in_maps, list(range(N_CORES)))
    out = np.stack([res.results[i]["out"] for i in range(N_CORES)])
    return out.reshape(B, C, H, W)


# revision 4
# speedup vs baseline: 2.4062x; 1.7711x over previous
"""AttentionBlock (GroupNorm + single-head attention over HW tokens + proj +
residual) as a Bass/Tile kernel for 8 Trainium2 NeuronCores.

Sharding: data-parallel over batch B=32 -> 4 samples per core; weights
replicated; no collectives. Full inputs in, full output out.

Core ideas (C=256, HW=1024 per sample):
  - All large matmuls use fp8 DoubleRow (contract 2x128 K per instruction at
    0.5 cycles/row, ~4x the f32r rate). Weights are folded on the host into
    [128, 2, N] DoubleRow layouts.
  - Scores use the algebraic fold q^T k = h^T (Wq^T Wk) h with L = Wk^T Wq
    precomputed on the host (fp8e4, upscaled 4x against subnormals and
    compensated in the exp scale), so only ONE projection (t = L^T h) needs
    a PSUM drain instead of two (q and k).
  - qkv biases fold exactly into softmax: the per-query term cancels in
    p/dn, the per-key term u = h^T(Wk^T bq) enters via the exp bias column.
    The v bias folds into b_proj on the host.
  - exp(s/16 - 2): 1/sqrt(C) lives in the activation scale, the -2 shift
    keeps fp8 in range. Six j-chunks on ACT (table exp -> e4m3), two on DVE
    via a Schraudolph int8 bit-trick writing e5m2 bytes directly (HW
    converts round-to-nearest).
  - dn via ones-lhsT DoubleRow matmuls over p (partition-axis reduction on
    the PE); 1/dn staged in SBUF; ao normalized at its drain.
  - ao and the projection run in f32r (removes two fp8 quantization stages
    on the worst-case peaked-softmax rows); residual is added via an
    identity f32r matmul into the proj PSUM; ACT drains with the b_proj
    bias and DMAs out.
  - GroupNorm stats via bn_stats/bn_aggr one-pass; rstd via Newton rsqrt
    split across DVE and Pool; h = x*sc+sh split Pool/DVE (fp8 out).
  - Software pipeline: sample i's epilogue is deferred past sample i+1's
    scores emission; stats run 2 samples ahead, x DMA 3 ahead. PSUM is
    managed as static rings (b2 x3, dn x1) within the 8 banks.
"""

import numpy as np
import ml_dtypes

import concourse.bacc as bacc
import concourse.tile as tile
import concourse.mybir as mybir
from concourse.bass_utils import run_bass_kernel_spmd

F32 = mybir.dt.float32
F32R = mybir.dt.float32r
F8E4 = mybir.dt.float8e4
F8E5 = mybir.dt.float8e5
I8 = mybir.dt.int8
I32 = mybir.dt.int32
ALU = mybir.AluOpType
ACTF = mybir.ActivationFunctionType
DR = mybir.MatmulPerfMode.DoubleRow

N_CORES = 8
B, C, H, W = 32, 256, 32, 32
HW = H * W          # 1024
S = B // N_CORES    # 4 samples per core
G = 8               # groups
CG = C // G         # 32 channels per group
EPS = 1e-5
NJ = HW // 128      # 8 j-chunks
NP = NJ // 2        # 4 j-chunk pairs

SCL = 0.0625        # 1/sqrt(C)
# Schraudolph constants for e5m2 output bytes: byte = A*s + dbias
SCH_A = (4.0 / np.log(2.0)) * SCL   # for the (unscaled) u bias column
SCH_AS = SCH_A / 4                  # for raw scores (L carries a 4x upscale)
SCH_B = 60.0 - 0.169 - 2.0 * (4.0 / np.log(2.0))
# jp PAIRS handled by DVE Schraudolph e5m2 (rest: ACT table exp, e4m3)
EXP_DVE_PAIRS = (1,)


def _emit_stats(nc, pools, wt, s, x_ap, xt=None, post_x_cb=None):
    """x DMA (unless a prefetched tile is passed) + GroupNorm stats down to
    per-channel scale/shift columns."""
    sb, ps = pools

    if xt is None:
        x_t = sb.tile([128, 2, HW], F32R, name=f"x_s{s}", tag="x", bufs=4)
        src = x_ap[s].rearrange("(t p) f -> p t f", t=2)
        if s == 0:
            nc.sync.dma_start(x_t[:, :, 0:512], src[:, :, 0:512])
            nc.sync.dma_start(x_t[:, :, 512:HW], src[:, :, 512:HW])
        else:
            nc.sync.dma_start(x_t[:], src)
    else:
        x_t = xt
    if post_x_cb is not None:
        post_x_cb()

    # per-channel stats: bn_stats halves -> bn_aggr -> [mean, E[x^2]] cols
    st = []
    for ci in range(2):
        bst = sb.tile([128, 2, 6], F32, name=f"bst_s{s}c{ci}", tag=f"bst{ci}",
                      bufs=2)
        nc.vector.bn_stats(bst[:, 0, :], x_t[:, ci, 0:512].bitcast(F32))
        nc.vector.bn_stats(bst[:, 1, :], x_t[:, ci, 512:HW].bitcast(F32))
        bag = sb.tile([128, 2], F32, name=f"bag_s{s}c{ci}", tag=f"bag{ci}",
                      bufs=2)
        nc.vector.bn_aggr(bag[:], bst[:])
        # col1 := mean*mean + var  (= E[x^2])
        nc.vector.tensor_scalar(bag[:, 1:2], bag[:, 0:1], bag[:, 0:1],
                                bag[:, 1:2], op0=ALU.mult, op1=ALU.add)
        st.append(bag)

    # group stats (gmask carries 1/CG): gst = [mean_g, E2_g]
    tiny = ps.tile([128, 2, 512], F32, name=f"tiny_s{s}", tag="b2", bufs=3)
    gst = tiny[0:8, 0, 0:2]
    for ci in range(2):
        nc.tensor.matmul(gst, wt["gmask"][:, ci * G:(ci + 1) * G], st[ci][:],
                         start=(ci == 0), stop=(ci == 1))
    gsb = sb.tile([8, 2], F32, name=f"gsb_s{s}", tag="gsb", bufs=2)
    nc.vector.tensor_copy(gsb[:], gst)
    msq = sb.tile([8, 1], F32, name=f"msq_s{s}", tag="msq", bufs=2)
    nc.gpsimd.tensor_mul(msq[:], gsb[:, 0:1], gsb[:, 0:1])
    var = sb.tile([8, 1], F32, name=f"var_s{s}", tag="var", bufs=2)
    nc.vector.scalar_tensor_tensor(var[:], in0=gsb[:, 1:2], scalar=EPS,
                                   in1=msq[:], op0=ALU.add, op1=ALU.subtract)
    # rstd = rsqrt(var): bit-trick seed + 2 Newton steps (all DVE, tiny)
    ish = sb.tile([8, 1], I32, name=f"ish_s{s}", tag="ish", bufs=2)
    nc.vector.tensor_scalar(ish[:], var[:].bitcast(I32), 1, None,
                            op0=ALU.arith_shift_right)
    yib = sb.tile([8, 1], I32, name=f"yib_s{s}", tag="yib", bufs=2)
    nc.gpsimd.tensor_tensor(yib[:], wt["magic"][0:8, :].bitcast(I32),
                            ish[:], op=ALU.subtract)
    y = yib[:].bitcast(F32)
    for it in range(2):
        ta = sb.tile([8, 1], F32, name=f"ta{it}_s{s}", tag=f"ta{it}", bufs=2)
        nc.gpsimd.tensor_mul(ta[:], y, y)
        tb = sb.tile([8, 1], F32, name=f"tb{it}_s{s}", tag=f"tb{it}", bufs=2)
        nc.gpsimd.tensor_mul(tb[:], ta[:], var[:])
        tcr = sb.tile([8, 1], F32, name=f"tc{it}_s{s}", tag=f"tc{it}", bufs=2)
        nc.gpsimd.tensor_scalar(tcr[:], tb[:], -0.5, 1.5, op0=ALU.mult,
                                op1=ALU.add)
        yn = sb.tile([8, 1], F32, name=f"yn{it}_s{s}", tag=f"yn{it}", bufs=2)
        nc.gpsimd.tensor_mul(yn[:], y, tcr[:])
        y = yn[:]
    gv2 = sb.tile([8, 2], F32, name=f"gv2_s{s}", tag="gv2", bufs=2)
    nc.gpsimd.tensor_copy(gv2[:, 0:1], y)
    nc.gpsimd.tensor_mul(gv2[:, 1:2], y, gsb[:, 0:1])

    # per-channel [gamma*rstd, gamma*mean*rstd] -> scale/shift cols in SBUF
    ssc = sb.tile([128, 2, 2], F32, name=f"ssc_s{s}", tag="ssc", bufs=3)
    for ci in range(2):
        mr = tiny[:, 0, 4 + 4 * ci:6 + 4 * ci]
        nc.tensor.matmul(mr, wt["maskTg"][:, ci * 128:(ci + 1) * 128],
                         gv2[:], start=True, stop=True)
        nc.vector.tensor_copy(ssc[:, ci, 0:1], mr[:, 0:1])
        nc.vector.tensor_sub(ssc[:, ci, 1:2], wt["beta"][:, ci, :], mr[:, 1:2])
    return x_t, ssc


def _emit_h(nc, pools, wt, s, stats):
    """h = x*sc + sh -> fp8e4. Chunk 0 runs on Pool (two TT passes with
    broadcast columns; Pool has no ptr-scalar op), chunk 1 on DVE."""
    sb, ps = pools
    x_t, ssc = stats
    h_t = sb.tile([128, 2, HW], F8E4, name=f"h_s{s}", tag="h", bufs=2)
    hm = sb.tile([128, HW], F32, name=f"hm_s{s}", tag="hm", bufs=2)
    nc.gpsimd.tensor_tensor(hm[:], x_t[:, 0, :].bitcast(F32),
                            ssc[:, 0, 0:1].to_broadcast((128, HW)),
                            op=ALU.mult)
    nc.gpsimd.tensor_tensor(h_t[:, 0, :], hm[:],
                            ssc[:, 0, 1:2].to_broadcast((128, HW)),
                            op=ALU.add)
    nc.vector.tensor_scalar(h_t[:, 1, :], x_t[:, 1, :].bitcast(F32),
                            ssc[:, 1, 0:1], ssc[:, 1, 1:2],
                            op0=ALU.mult, op1=ALU.add)
    return x_t, h_t


def _emit_qkv_t_mm(nc, pools, wt, s, h_t):
    """t = L^T h matmuls into PSUM."""
    sb, ps = pools
    t_sb = sb.tile([128, 2, HW], F8E4, name=f"t_s{s}", tag="t", bufs=2)
    tps = []
    for ci in range(2):
        tp = ps.tile([128, 2, 512], F32, name=f"tp_s{s}c{ci}", tag="b2",
                     bufs=3)
        for ih in range(2):
            hs = slice(ih * 512, (ih + 1) * 512)
            nc.tensor.matmul(tp[:, ih, :],
                             wt["L"][:, :, ci * 128:(ci + 1) * 128],
                             h_t[:, :, hs], start=True, stop=True,
                             perf_mode=DR)
        tps.append(tp)
    return t_sb, tps


def _emit_qkv_t_drain(nc, pools, wt, t_sb, tps):
    """t PSUM -> fp8 SBUF; the two chunks drain on different engines so
    they run in parallel (t gates the next sample's scores)."""
    nc.scalar.copy(t_sb[:, 0, :], tps[0][:].rearrange("p a b -> p (a b)"))
    nc.vector.tensor_copy(t_sb[:, 1, :],
                          tps[1][:].rearrange("p a b -> p (a b)"))


def _emit_qkv_v(nc, pools, wt, s, h_t):
    """vT (fp8e4, (HW,C) chunk-pair layout); drain on DVE (off critical
    path, emitted late)."""
    sb, ps = pools
    v_sb = []
    for vp2 in range(2):
        vp = ps.tile([128, 2, 512], F32, name=f"vp_s{s}p{vp2}", tag="b2",
                     bufs=3)
        vpv = vp[:].rearrange("p a (c d) -> p (a c) d", c=2)
        for sub in range(4):
            j = vp2 * 4 + sub
            nc.tensor.matmul(vpv[:, sub, :],
                             h_t[:, :, j * 128:(j + 1) * 128],
                             wt["Wv"][:], start=True, stop=True, perf_mode=DR)
        v_t = sb.tile([128, 4, 256], F8E4, name=f"v_s{s}p{vp2}",
                      tag=f"v{vp2}", bufs=2)
        if vp2 == 0:
            nc.scalar.copy(v_t[:], vpv)
        else:
            nc.vector.tensor_copy(v_t[:], vpv)
        v_sb.append(v_t)
    return v_sb


def _emit_ubias(nc, pools, wt, s, h_t):
    """u[j] = h^T (Wk^T bq) tiny matmuls + the two exp bias column sets."""
    sb, ps = pools
    up = ps.tile([128, 2, 512], F32, name=f"up_s{s}", tag="b2", bufs=3)
    for j in range(NJ):
        nc.tensor.matmul(up[:, 0, j:j + 1],
                         h_t[:, :, j * 128:(j + 1) * 128],
                         wt["wkbq"][:], start=True, stop=True, perf_mode=DR)
    ebias = sb.tile([128, 2 * NJ], F32, name=f"eb_s{s}", tag="eb", bufs=2)
    nc.vector.tensor_scalar(ebias[:, 0:NJ], up[:, 0, 0:NJ], SCL, -2.0,
                            op0=ALU.mult, op1=ALU.add)
    nc.vector.tensor_scalar(ebias[:, NJ:2 * NJ], up[:, 0, 0:NJ], SCH_A,
                            SCH_B, op0=ALU.mult, op1=ALU.add)
    p_sb = [sb.tile([128, 2, HW],
                    F8E5 if jp in EXP_DVE_PAIRS else F8E4,
                    name=f"p_s{s}j{jp}", tag=f"p{jp}", bufs=2)
            for jp in range(NP)]
    return ebias, p_sb


def _emit_scores_j(nc, pools, wt, s, h_t, t_sb, ebias, p_sb, js):
    """Scores + exp for j-chunks in js."""
    sb, ps = pools
    for j in js:
        sp = ps.tile([128, 2, 512], F32, name=f"sp_s{s}j{j}", tag="b2",
                     bufs=3)
        for ih in range(2):
            hs = slice(ih * 512, (ih + 1) * 512)
            nc.tensor.matmul(sp[:, ih, :],
                             t_sb[:, :, j * 128:(j + 1) * 128],
                             h_t[:, :, hs], start=True, stop=True,
                             perf_mode=DR)
        spf = sp[:].rearrange("p a b -> p (a b)")
        pdst = p_sb[j // 2][:, j % 2, :]
        if j // 2 in EXP_DVE_PAIRS:
            nc.vector.tensor_scalar(pdst.bitcast(I8), spf, SCH_AS,
                                    ebias[:, NJ + j:NJ + j + 1],
                                    op0=ALU.mult, op1=ALU.add)
        else:
            nc.scalar.activation(pdst, spf, ACTF.Exp,
                                 bias=ebias[:, j:j + 1], scale=wt["sclcol"])


def _emit_dn(nc, pools, wt, s, p_sb):
    sb, ps = pools
    dn = ps.tile([128, 2, 512], F32, name=f"dn_s{s}", tag="dn", bufs=1)
    for ih in range(2):
        hs = slice(ih * 512, (ih + 1) * 512)
        for jp in range(NP):
            nc.tensor.matmul(dn[:, ih, :],
                             wt["ones5" if jp in EXP_DVE_PAIRS
                                else "ones4"][:],
                             p_sb[jp][:, :, hs],
                             start=(jp == 0), stop=(jp == NP - 1),
                             perf_mode=DR)
    # HW allows only one PSUM operand per DVE op: stage dn in SBUF for the
    # divides (also releases the dn bank early)
    rb = sb.tile([128, 2, 512], F32, name=f"rb_s{s}", tag="rb", bufs=2)
    nc.vector.reciprocal(rb[:], dn[:])
    return rb


def _emit_out_ih(nc, pools, wt, s, ih, x_t, p_sb, v_sb, dn, out_ap):
    """One query-half: ao (/dn) -> proj + residual + bias -> DMA."""
    sb, ps = pools
    hs = slice(ih * 512, (ih + 1) * 512)
    ao = ps.tile([128, 2, 512], F32, name=f"ao_s{s}h{ih}", tag="b2",
                 bufs=3)
    for ci in range(2):
        for jp in range(NP):
            nc.tensor.matmul(
                ao[:, ci, :],
                v_sb[jp // 2][:, 2 * (jp % 2):2 * (jp % 2) + 2,
                              ci * 128:(ci + 1) * 128],
                p_sb[jp][:, :, hs],
                start=(jp == 0), stop=(jp == NP - 1), perf_mode=DR)
    ao_sb = sb.tile([128, 2, 512], F32R, name=f"aos_s{s}h{ih}",
                    tag="aos", bufs=3)
    for ci in range(2):
        nc.vector.tensor_mul(ao_sb[:, ci, :], ao[:, ci, :], dn[:, ih, :])

    pp = ps.tile([128, 2, 512], F32, name=f"pp_s{s}h{ih}", tag="b2",
                 bufs=3)
    for ci in range(2):
        for cc in range(2):
            nc.tensor.matmul(pp[:, ci, :],
                             wt["Wp"][:, cc, ci * 128:(ci + 1) * 128],
                             ao_sb[:, cc, :], start=(cc == 0), stop=False)
        nc.tensor.matmul(pp[:, ci, :], wt["I128"][:],
                         x_t[:, ci, hs],
                         start=False, stop=True)
    o_sb = sb.tile([128, 2, 512], F32, name=f"o_s{s}h{ih}",
                   tag=f"o{ih}", bufs=2)
    for ci in range(2):
        nc.scalar.add(o_sb[:, ci, :], pp[:, ci, :], wt["bp"][:, ci, :])
    nc.sync.dma_start(
        out_ap[s, :, hs].rearrange("(t p) f -> p t f", t=2), o_sb[:])


def build_program(reps=1):
    nc = bacc.Bacc("TRN2", target_bir_lowering=False, debug=False,
                   enable_asserts=False, num_devices=N_CORES)

    x_ap = nc.dram_tensor("x", [S, C, HW], F32R, kind="ExternalInput").ap()
    L_ap = nc.dram_tensor("L", [128, 2, C], F8E4, kind="ExternalInput").ap()
    wv_ap = nc.dram_tensor("Wv", [128, 2, C], F8E4, kind="ExternalInput").ap()
    wp_ap = nc.dram_tensor("Wp", [128, 2, C], F32R, kind="ExternalInput").ap()
    wkbq_ap = nc.dram_tensor("wkbq", [128, 2, 1], F8E4,
                             kind="ExternalInput").ap()
    o4_ap = nc.dram_tensor("ones4", [128, 2, 128], F8E4,
                           kind="ExternalInput").ap()
    o5_ap = nc.dram_tensor("ones5", [128, 2, 128], F8E5,
                           kind="ExternalInput").ap()
    eye_ap = nc.dram_tensor("eye", [128, 128], F32R, kind="ExternalInput").ap()
    ca_ap = nc.dram_tensor("constsA", [128, 2 * G + 8], F32,
                           kind="ExternalInput").ap()
    gmt_ap = nc.dram_tensor("gmaskTg", [G, C], F32, kind="ExternalInput").ap()
    out_ap = nc.dram_tensor("out", [S, C, HW], F32, kind="ExternalOutput").ap()

    with tile.TileContext(nc) as tc:
        with (
            tc.tile_pool(name="wpool", bufs=1) as wp,
            tc.tile_pool(name="sb", bufs=2) as sb,
            tc.tile_pool(name="ps", bufs=2, space="PSUM") as ps,
        ):
            constsA = wp.tile([128, 2 * G + 8], F32, name="constsA",
                              tag="constsA")
            nc.sync.dma_start(constsA[:], ca_ap[:])
            maskTg = wp.tile([G, C], F32, name="maskTg", tag="maskTg")
            eye = wp.tile([128, 128], F32R, name="eye", tag="eye")
            ones4 = wp.tile([128, 2, 128], F8E4, name="ones4", tag="ones4")
            ones5 = wp.tile([128, 2, 128], F8E5, name="ones5", tag="ones5")
            L_t = wp.tile([128, 2, C], F8E4, name="L", tag="L")
            wv_t = wp.tile([128, 2, C], F8E4, name="Wv", tag="Wv")
            wp_t = wp.tile([128, 2, C], F32R, name="Wp", tag="Wp")
            wkbq = wp.tile([128, 2, 1], F8E4, name="wkbq", tag="wkbq")

            wt = {
                "gmask": constsA[:, 0:2 * G],
                "maskTg": maskTg,
                "magic": constsA[:, 2 * G:2 * G + 1],
                "sclcol": constsA[:, 2 * G + 1:2 * G + 2],
                "s16col": constsA[:, 2 * G + 6:2 * G + 7],
                "beta": constsA[:, 2 * G + 2:2 * G + 4]
                        .rearrange("p (t o) -> p t o", t=2),
                "bp": constsA[:, 2 * G + 4:2 * G + 6]
                        .rearrange("p (t o) -> p t o", t=2),
                "L": L_t, "Wv": wv_t, "Wp": wp_t, "wkbq": wkbq,
                "ones4": ones4, "ones5": ones5, "I128": eye,
            }

            pools = (sb, ps)
            stats = {}
            stats[0] = _emit_stats(
                nc, pools, wt, 0, x_ap,
                post_x_cb=lambda: (nc.sync.dma_start(maskTg[:], gmt_ap[:]),
                                   nc.sync.dma_start(eye[:], eye_ap[:]),
                                   nc.sync.dma_start(ones4[:], o4_ap[:]),
                                   nc.sync.dma_start(ones5[:], o5_ap[:]),
                                   nc.sync.dma_start(wkbq[:], wkbq_ap[:])))
            stats[1] = _emit_stats(nc, pools, wt, 1, x_ap)
            nc.sync.dma_start(L_t[:], L_ap[:])
            nc.sync.dma_start(wv_t[:], wv_ap[:])
            nc.sync.dma_start(wp_t[:], wp_ap[:])

            seq = [(rep, s) for rep in range(reps) for s in range(S)]
            n_seq = len(seq)
            h_cur = _emit_h(nc, pools, wt, 0, stats.pop(0))
            t_sb0, tps0 = _emit_qkv_t_mm(nc, pools, wt, 0, h_cur[1])
            _emit_qkv_t_drain(nc, pools, wt, t_sb0, tps0)
            v0 = _emit_qkv_v(nc, pools, wt, 0, h_cur[1])

            state = {"h": h_cur, "t": t_sb0, "v": v0}
            pend = None
            for i in range(n_seq):
                s = seq[i][1]
                x_t, h_t = state["h"]
                t_sb, v_sb = state["t"], state["v"]

                # 1. exp bias columns
                ebias, p_sb = _emit_ubias(nc, pools, wt, s, h_t)

                # 2. next sample's h (Pool + DVE early in their queues)
                nh = None
                if i + 1 < n_seq:
                    nh = _emit_h(nc, pools, wt, seq[i + 1][1],
                                 stats.pop(i + 1))
                    state["h"] = nh

                # 3. scores + exp j0..j7
                _emit_scores_j(nc, pools, wt, s, h_t, t_sb, ebias, p_sb,
                               range(NJ))

                # 4. previous sample's epilogue (ao/mult/pp after exp in
                # queues; its psum ring slots free as exp drains them)
                if pend is not None:
                    _emit_out_ih(nc, pools, wt, pend["s"], 0, pend["x"],
                                 pend["p"], pend["v"], pend["rb"], out_ap)
                    _emit_out_ih(nc, pools, wt, pend["s"], 1, pend["x"],
                                 pend["p"], pend["v"], pend["rb"], out_ap)
                    pend = None

                # 5. next sample's t matmuls + ACT drains (after exp_i)
                if i + 1 < n_seq:
                    nt, ntps = _emit_qkv_t_mm(nc, pools, wt, seq[i + 1][1],
                                              nh[1])
                    _emit_qkv_t_drain(nc, pools, wt, nt, ntps)
                    state["t"] = nt

                # 6. dn + reciprocal for sample i
                rb = _emit_dn(nc, pools, wt, s, p_sb)

                # 7. next sample's v
                if i + 1 < n_seq:
                    state["v"] = _emit_qkv_v(nc, pools, wt, seq[i + 1][1],
                                             nh[1])

                # 8. stats for i+2 + x DMA for i+3
                if i + 2 < n_seq:
                    stats[i + 2] = _emit_stats(
                        nc, pools, wt, seq[i + 2][1], x_ap,
                        xt=stats.pop(("xt", i + 2), None))
                if i + 3 < n_seq:
                    xs = seq[i + 3][1]
                    xt3 = sb.tile([128, 2, HW], F32R, name=f"x_s{xs}",
                                  tag="x", bufs=4)
                    nc.sync.dma_start(
                        xt3[:],
                        x_ap[xs].rearrange("(t p) f -> p t f", t=2))
                    stats[("xt", i + 3)] = xt3

                pend = {"s": s, "x": x_t, "p": p_sb, "v": v_sb, "rb": rb}

            _emit_out_ih(nc, pools, wt, pend["s"], 0, pend["x"], pend["p"],
                         pend["v"], pend["rb"], out_ap)
            _emit_out_ih(nc, pools, wt, pend["s"], 1, pend["x"], pend["p"],
                         pend["v"], pend["rb"], out_ap)

    nc.compile()
    return nc


def _f8(x, dt=ml_dtypes.float8_e4m3):
    return np.asarray(x, np.float32).astype(dt)


def _fold(mat):
    """(256, N) -> [128, 2, N] DoubleRow contraction layout."""
    n = mat.shape[1]
    return np.ascontiguousarray(mat.reshape(2, 128, n).transpose(1, 0, 2))


def prep_inputs(x, gamma, beta, w_qkv, b_qkv, w_proj, b_proj):
    x = np.ascontiguousarray(x, dtype=np.float32).reshape(B, C, HW)
    x_shards = x.reshape(N_CORES, S, C, HW)

    w_qkv = np.asarray(w_qkv, np.float32)
    b_qkv = np.asarray(b_qkv, np.float32)
    w_proj = np.asarray(w_proj, np.float32)
    b_proj = np.asarray(b_proj, np.float32)
    gamma = np.asarray(gamma, np.float32)
    beta = np.asarray(beta, np.float32)

    Wq, Wk, Wv = w_qkv[0:C], w_qkv[C:2 * C], w_qkv[2 * C:3 * C]
    L = Wk.T @ Wq                      # L[b, a]
    wkbq = Wk.T @ b_qkv[0:C]           # per-j score bias term
    bp_eff = b_proj + w_proj @ b_qkv[2 * C:3 * C]

    # constsA: gmask (1/CG for bn-path) | magic | scale | beta cols | bp cols
    gmask = np.zeros((128, 2 * G), np.float32)
    for c in range(C):
        gmask[c % 128, (c // 128) * G + c // CG] = 1.0 / CG
    cvec = np.zeros((128, 8), np.float32)
    cvec[:, 0] = np.uint32(0x5F3759DF).view(np.float32)
    cvec[:, 1] = SCL / 4
    cvec[:, 2:4] = beta.reshape(2, 128).T
    cvec[:, 4:6] = bp_eff.reshape(2, 128).T
    cvec[:, 6] = 1.0 / 16

    gmaskTg = np.zeros((G, C), np.float32)
    for c in range(C):
        gmaskTg[c // CG, c] = gamma[c]

    shared = {
        "L": _f8(_fold(L * 4.0)),
        "Wv": _f8(_fold(Wv.T * 4.0)),
        "Wp": _fold(w_proj.T).astype(np.float32),
        "wkbq": _f8(_fold(wkbq.reshape(C, 1))),
        "ones4": np.full((128, 2, 128), 4.0, ml_dtypes.float8_e4m3),
        "ones5": np.full((128, 2, 128), 4.0, ml_dtypes.float8_e5m2),
        "eye": np.eye(128, dtype=np.float32),
        "constsA": np.ascontiguousarray(np.concatenate([gmask, cvec], 1)),
        "gmaskTg": gmaskTg,
    }
    return [dict(shared, x=np.ascontiguousarray(x_shards[i]))
            for i in range(N_CORES)]


_NC_CACHE = {}


def kernel(x, gamma, beta, w_qkv, b_qkv, w_proj, b_proj):
    if "nc" not in _NC_CACHE:
        _NC_CACHE["nc"] = build_program()
    nc = _NC_CACHE["nc"]
    in_maps = prep_inputs(x, gamma, beta, w_qkv, b_qkv, w_proj, b_proj)
    res = run_bass_kernel_spmd(nc, in_maps, list(range(N_CORES)))
    out = np.stack([res.results[i]["out"] for i in range(N_CORES)])
    return out.reshape(B, C, H, W)
